# revision 1
# baseline (speedup 1.0000x reference)
"""Trainium2 Bass kernel for the annealed mean-field Boltzmann machine.

Strategy: 1D tensor-parallel over 8 NeuronCores. Each core holds a
256-column shard of hh/vis_hid and a 512-column shard of vv/vis_hid.T,
all SBUF-resident in fp32 (the dynamics are chaotic in the early
annealing steps, so reduced-precision matmuls diverge; fp32 matches the
reference to the level of summation-order noise).

States are kept transposed (feature-on-partition, batch-on-free).
Every field matmul uses the weight tile as the stationary operand
(128x128) and a state k-tile (128x64) as the moving operand:
out[feat_tile, batch] += W[k, feat_tile].T @ stateT[k]. Outputs come
out feature-major, exactly the layout the next step needs, so there are
no transposes anywhere. Bias enters as a rank-1 matmul (bias x ones),
sigmoid/(1/temp) on the scalar engine, 0.9/0.1 mixing on the vector
engine, and each core's state shard is AllGathered so every core has
the full state for the next half-step.
"""

import sys
import time

sys.path.insert(0, "/opt/trn_rl_repo")

import numpy as np

N_CORES = 8
V_SIZE = 4096
H_SIZE = 2048
BATCH = 64
HS = H_SIZE // N_CORES  # 256 hid cols per core
VS = V_SIZE // N_CORES  # 512 vis cols per core
KT_H = H_SIZE // 128  # 16 k-tiles over hid features
KT_V = V_SIZE // 128  # 32 k-tiles over vis features
NTH = HS // 128  # 2 feature out-tiles per core (hid)
NTV = VS // 128  # 4 feature out-tiles per core (vis)

_BUILT = {}


def _build(n_steps: int, temps: np.ndarray, sim_mode: bool = False,
           no_comm: bool = False, splits=(3, (2, 6, 8), (2, 6, 8))):
    import concourse.bacc as bacc
    import concourse.tile as tile
    import concourse.mybir as mybir

    F32 = mybir.dt.float32
    SIG = mybir.ActivationFunctionType.Sigmoid
    MULT = mybir.AluOpType.mult
    ADD = mybir.AluOpType.add

    nc = bacc.Bacc(
        "TRN2",
        target_bir_lowering=False,
        debug=False,
        enable_asserts=True,
        num_devices=1 if sim_mode else N_CORES,
    )

    def din(name, shape):
        return nc.dram_tensor(name, shape, F32, kind="ExternalInput").ap()

    xT = din("xT", [V_SIZE, BATCH])
    xT_my = din("xT_my", [VS, BATCH])
    hid0T = din("hid0T", [H_SIZE, BATCH])
    hh_w = din("hh_w", [H_SIZE, HS])
    vh_w = din("vh_w", [V_SIZE, HS])
    vv_w = din("vv_w", [V_SIZE, VS])
    vht_w = din("vht_w", [H_SIZE, VS])
    hb_row = din("hb_row", [1, HS])
    vb_row = din("vb_row", [1, VS])
    ones_row = din("ones_row", [1, BATCH])
    out_vis = nc.dram_tensor(
        "vis_shT", [VS, BATCH], F32, kind="ExternalOutput"
    ).ap()

    rg = [list(range(N_CORES))]
    shared_as = "Local" if sim_mode else "Shared"

    def all_gather(ag_in, ag_out, scratch):
        """Real AllGather, or in sim mode a 2-DMA pool-engine chain through a
        scratch DRAM tile: same ~5us latency and a single completion, without
        touching the engines/queues the real collective leaves free."""
        if not sim_mode:
            nc.gpsimd.collective_compute(
                "AllGather",
                mybir.AluOpType.bypass,
                replica_groups=rg,
                ins=[ag_in[:].opt()],
                outs=[ag_out[:].opt()],
            )
        else:
            rows = ag_in.shape[0]
            nc.gpsimd.dma_start(scratch[0:rows, :], ag_in[:])
            nc.gpsimd.dma_start(ag_out[:], scratch[:])

    with tile.TileContext(nc) as tc:
        with (
            tc.tile_pool(name="w", bufs=1) as wpool,
            tc.tile_pool(name="st", bufs=1) as stpool,
            tc.tile_pool(name="act", bufs=3) as actpool,
            tc.tile_pool(name="ps_h", bufs=2, space="PSUM") as ps_h,
            tc.tile_pool(name="ps_v", bufs=4, space="PSUM") as ps_v,
            tc.tile_pool(name="dram", bufs=2, space="DRAM") as dram,
        ):
            # --- weights (SBUF-resident), blocked [k, j] 128x128 ---
            hh_sb = wpool.tile([128, KT_H, NTH, 128], F32)
            vh_sb = wpool.tile([128, KT_V, NTH, 128], F32)
            vv_sb = wpool.tile([128, KT_V, NTV, 128], F32)
            vht_sb = wpool.tile([128, KT_H, NTV, 128], F32)
            for j in range(0, KT_H, 4):
                nc.sync.dma_start(
                    hh_sb[:, j : j + 4, :, :],
                    hh_w.rearrange("(k p) (j n) -> p k j n", p=128, n=128)[
                        :, j : j + 4, :, :
                    ],
                )
                nc.sync.dma_start(
                    vht_sb[:, j : j + 4, :, :],
                    vht_w.rearrange("(k p) (j n) -> p k j n", p=128, n=128)[
                        :, j : j + 4, :, :
                    ],
                )
            for j in range(0, KT_V, 4):
                nc.sync.dma_start(
                    vh_sb[:, j : j + 4, :, :],
                    vh_w.rearrange("(k p) (j n) -> p k j n", p=128, n=128)[
                        :, j : j + 4, :, :
                    ],
                )
                nc.sync.dma_start(
                    vv_sb[:, j : j + 4, :, :],
                    vv_w.rearrange("(k p) (j n) -> p k j n", p=128, n=128)[
                        :, j : j + 4, :, :
                    ],
                )

            # --- constants ---
            hb_sb = wpool.tile([1, HS], F32)
            vb_sb = wpool.tile([1, VS], F32)
            ones_sb = wpool.tile([1, BATCH], F32)
            nc.sync.dma_start(hb_sb[:], hb_row[:])
            nc.sync.dma_start(vb_sb[:], vb_row[:])
            nc.sync.dma_start(ones_sb[:], ones_row[:])

            # --- states (transposed: feature-on-partition) ---
            visT = stpool.tile([128, KT_V, BATCH], F32)
            hidT = stpool.tile([128, KT_H, BATCH], F32)
            vmyT = stpool.tile([128, NTV, BATCH], F32)
            hmyT = stpool.tile([128, NTH, BATCH], F32)
            for j in range(0, KT_V, 8):
                nc.sync.dma_start(
                    visT[:, j : j + 8, :],
                    xT.rearrange("(k p) n -> p k n", p=128)[:, j : j + 8, :],
                )
            nc.sync.dma_start(vmyT[:], xT_my.rearrange("(k p) n -> p k n", p=128))
            nc.sync.dma_start(hidT[:], hid0T.rearrange("(k p) n -> p k n", p=128))
            nc.sync.dma_start(
                hmyT[:],
                hid0T.rearrange("(k p) n -> p k n", p=128)[:, :NTH, :],
            )

            # vis and hid k-tiles are stored in AllGather output order
            # (host-side permutation), so restage is a contiguous copy and
            # consuming k in ascending order reads the early half first
            scr_h0 = dram.tile([H_SIZE, BATCH], F32, name="scr_h0", bufs=1)
            scr_h1 = dram.tile([H_SIZE // 2, BATCH], F32, name="scr_h1", bufs=1)
            scr_v0 = dram.tile([H_SIZE, BATCH], F32, name="scr_v0", bufs=1)
            scr_v1 = dram.tile([H_SIZE, BATCH], F32, name="scr_v1", bufs=1)
            scr_h = [scr_h0, scr_h1]
            scr_v = [scr_v0, scr_v1]
            KH2 = KT_H // 2  # 8: k-tiles per hid AllGather half
            KV2 = KT_V // 2  # 16: k-tiles per vis AllGather half
            # restage chunk ladder: small first chunk lands fast so the
            # first consumer matmuls start early
            RESTAGE_H = splits[1] if splits[1] else (2, 14)
            RESTAGE_V = splits[2] if splits[2] else (2, 14)

            for i in range(n_steps):
                inv_t = float(1.0 / temps[i])
                last = i == n_steps - 1

                # ---- field matmuls, ordered so that every comm chain is
                # covered by matmuls that do not depend on it ----
                phs = []
                for j in range(NTH):
                    ph = ps_h.tile(
                        [128, BATCH], F32, name=f"ph{i}_{j}", tag="ph"
                    )
                    phs.append(ph)
                    nc.tensor.matmul(
                        ph[:],
                        hb_sb[:, j * 128 : (j + 1) * 128],
                        ones_sb[:],
                        start=True,
                        stop=False,
                    )
                pvs = []
                for j in range(NTV):
                    pv = ps_v.tile(
                        [128, BATCH], F32, name=f"pv{i}_{j}", tag="pv"
                    )
                    pvs.append(pv)
                    nc.tensor.matmul(
                        pv[:],
                        vb_sb[:, j * 128 : (j + 1) * 128],
                        ones_sb[:],
                        start=True,
                        stop=False,
                    )
                # hid @ hh (needs hidT(i-1): available early)
                for k in range(KT_H):
                    for j in range(NTH):
                        nc.tensor.matmul(
                            phs[j][:], hh_sb[:, k, j, :], hidT[:, k, :],
                            start=False, stop=False,
                        )
                # vis-dependent matmuls, A-half k-tiles then B-half, with the
                # vv-A block between them as cover for the late AG_v_B
                for k in range(KV2):
                    for j in range(NTH):
                        nc.tensor.matmul(
                            phs[j][:], vh_sb[:, k, j, :], visT[:, k, :],
                            start=False, stop=False,
                        )
                VV_A = splits[0]
                for k in range(VV_A):
                    for j in range(NTV):
                        nc.tensor.matmul(
                            pvs[j][:], vv_sb[:, k, j, :], visT[:, k, :],
                            start=False, stop=False,
                        )
                for k in range(KV2, KT_V):
                    for j in range(NTH):
                        nc.tensor.matmul(
                            phs[j][:], vh_sb[:, k, j, :], visT[:, k, :],
                            start=False, stop=(k == KT_V - 1),
                        )
                # hid field complete: sigmoid + mix + AllGather
                for j in range(NTH):
                    ph = phs[j]
                    prob = actpool.tile(
                        [128, BATCH], F32, name=f"prh{i}_{j}", tag="pr"
                    )
                    nc.scalar.activation(prob[:], ph[:], SIG, scale=inv_t)
                    tmp = actpool.tile(
                        [128, BATCH], F32, name=f"tmh{i}_{j}", tag="tm"
                    )
                    nc.vector.tensor_sub(tmp[:], prob[:], hmyT[:, j, :])
                    nc.vector.scalar_tensor_tensor(
                        hmyT[:, j, :], tmp[:], 0.1, hmyT[:, j, :], MULT, ADD
                    )
                    if not no_comm:
                        if j == 0:
                            ag_in_h = dram.tile(
                                [HS, BATCH], F32, name=f"agih{i}", tag="agih"
                            )
                        nc.scalar.dma_start(
                            ag_in_h[j * 128 : (j + 1) * 128, :],
                            hmyT[:, j, :],
                        )
                if not no_comm:
                    ag_out_h = dram.tile(
                        [H_SIZE, BATCH], F32, addr_space=shared_as,
                        name=f"agoh{i}", tag="agoh",
                    )
                    all_gather(ag_in_h, ag_out_h, scr_h[0])
                    qs = 0
                    for w in RESTAGE_H:
                        nc.sync.dma_start(
                            hidT[:, qs : qs + w, :],
                            ag_out_h[:].rearrange("(k p) n -> p k n", p=128)[
                                :, qs : qs + w, :
                            ],
                        )
                        qs += w
                # finish vv for j0/j1 (covers the hid AllGather)
                for k in range(VV_A, KT_V):
                    for j in (0, 1):
                        nc.tensor.matmul(
                            pvs[j][:], vv_sb[:, k, j, :], visT[:, k, :],
                            start=False, stop=False,
                        )
                # hid(i)-dependent part of the vis field, then mix + AG per
                # j-pair; the j2/j3 vv tail runs between the two halves as
                # cover for AG_v_A
                for half in range(2):
                    js = (0, 1) if half == 0 else (2, 3)
                    for k in range(KT_H):
                        for j in js:
                            nc.tensor.matmul(
                                pvs[j][:], vht_sb[:, k, j, :], hidT[:, k, :],
                                start=False, stop=(k == KT_H - 1),
                            )
                    if half == 0:
                        for k in range(VV_A, KT_V):
                            for j in (2, 3):
                                nc.tensor.matmul(
                                    pvs[j][:], vv_sb[:, k, j, :], visT[:, k, :],
                                    start=False, stop=False,
                                )
                    for j in js:
                        pv = pvs[j]
                        prob = actpool.tile(
                            [128, BATCH], F32, name=f"prv{i}_{j}", tag="pr"
                        )
                        nc.scalar.activation(prob[:], pv[:], SIG, scale=inv_t)
                        tmp = actpool.tile(
                            [128, BATCH], F32, name=f"tmv{i}_{j}", tag="tm"
                        )
                        nc.vector.tensor_sub(tmp[:], prob[:], vmyT[:, j, :])
                        nc.vector.scalar_tensor_tensor(
                            vmyT[:, j, :], tmp[:], 0.1, vmyT[:, j, :], MULT, ADD
                        )
                        if not (last or no_comm):
                            if j % 2 == 0:
                                ag_in = dram.tile(
                                    [HS, BATCH], F32,
                                    name=f"agiv{i}_{half}", tag="agiv",
                                )
                            nc.scalar.dma_start(
                                ag_in[(j % 2) * 128 : (j % 2 + 1) * 128, :],
                                vmyT[:, j, :],
                            )
                    if last or no_comm:
                        continue
                    ag_out = dram.tile(
                        [H_SIZE, BATCH], F32, addr_space=shared_as,
                        name=f"agov{i}_{half}", tag="agov",
                    )
                    all_gather(ag_in, ag_out, scr_v[half])
                    qs = 0
                    for w in RESTAGE_V:
                        nc.sync.dma_start(
                            visT[:, KV2 * half + qs : KV2 * half + qs + w, :],
                            ag_out[:].rearrange("(k p) n -> p k n", p=128)[
                                :, qs : qs + w, :
                            ],
                        )
                        qs += w

            nc.sync.dma_start(
                out_vis[:].rearrange("(k p) n -> p k n", p=128), vmyT[:]
            )

    nc.compile()
    return nc


# vis k-tile permutation: SBUF order k' = AllGather output order.
# k' in [0,16): half A = each core's feature tiles {0,1};  orig k = 4c+t
# k' in [16,32): half B = tiles {2,3};                      orig k = 4c+2+t
_PERM_V = [4 * (k % 16 // 2) + (2 * (k // 16)) + (k % 2) for k in range(32)]
# hid k-tile permutation: per-j AllGather j=0 gathers each core's tile 0
# (orig 2c) into k' = c, j=1 gathers tile 1 (orig 2c+1) into k' = 8+c
_PERM_H = [2 * k for k in range(8)] + [2 * k + 1 for k in range(8)]


def _permute_vis_rows(a):
    """Reorder 128-row blocks of a (4096, ...) array into gather order."""
    blocks = a.reshape(32, 128, *a.shape[1:])
    return np.ascontiguousarray(blocks[_PERM_V].reshape(a.shape))


def _permute_hid_rows(a):
    """Reorder 128-row blocks of a (2048, ...) array into gather order."""
    blocks = a.reshape(16, 128, *a.shape[1:])
    return np.ascontiguousarray(blocks[_PERM_H].reshape(a.shape))


def _prep_inputs(x, vis_bias, hid_bias, vis_hid, vis_vis_raw, hid_hid_raw):
    f32 = np.float32
    vv = np.triu(np.asarray(vis_vis_raw, dtype=f32), 1)
    vv = vv + vv.T
    hh = np.triu(np.asarray(hid_hid_raw, dtype=f32), 1)
    hh = hh + hh.T
    vis_hid = np.ascontiguousarray(np.asarray(vis_hid, dtype=f32))
    vht = np.ascontiguousarray(vis_hid.T)  # (H, V)
    x = np.asarray(x, dtype=f32)
    xT = np.ascontiguousarray(x.T)
    ones = np.ones((1, BATCH), dtype=f32)
    hid0 = np.full((H_SIZE, BATCH), 0.5, dtype=f32)
    hb = np.ascontiguousarray(np.asarray(hid_bias, dtype=f32).reshape(1, H_SIZE))
    vb = np.ascontiguousarray(np.asarray(vis_bias, dtype=f32).reshape(1, V_SIZE))

    in_maps = []
    for c in range(N_CORES):
        hsl = slice(c * HS, (c + 1) * HS)
        vsl = slice(c * VS, (c + 1) * VS)
        in_maps.append(
            {
                "xT": _permute_vis_rows(xT),
                "xT_my": np.ascontiguousarray(xT[vsl]),
                "hid0T": hid0,
                "hh_w": np.ascontiguousarray(hh[:, hsl]),
                "vh_w": _permute_vis_rows(np.ascontiguousarray(vis_hid[:, hsl])),
                "vv_w": _permute_vis_rows(np.ascontiguousarray(vv[:, vsl])),
                "vht_w": np.ascontiguousarray(vht[:, vsl]),
                "hb_row": np.ascontiguousarray(hb[:, hsl]),
                "vb_row": np.ascontiguousarray(vb[:, vsl]),
                "ones_row": ones,
            }
        )
    return in_maps


def kernel(
    x,
    vis_bias,
    hid_bias,
    vis_hid,
    vis_vis_raw,
    hid_hid_raw,
    max_steps,
    _trace=False,
):
    from concourse import bass_utils

    n_steps = int(max_steps)
    steps_f = np.float32(n_steps)
    temps = (
        np.float32(0.01)
        * (
            np.float32(1.0)
            + np.float32(4.0)
            * np.exp(
                np.float32(-5.0)
                * np.arange(n_steps, dtype=np.float32)
                / steps_f
            )
        )
    ).astype(np.float32)

    if n_steps not in _BUILT:
        _BUILT[n_steps] = _build(n_steps, temps)
    nc = _BUILT[n_steps]

    in_maps = _prep_inputs(
        x, vis_bias, hid_bias, vis_hid, vis_vis_raw, hid_hid_raw
    )
    res = bass_utils.run_bass_kernel_spmd(
        nc, in_maps, core_ids=list(range(N_CORES)), trace=_trace
    )

    out = np.empty((BATCH, V_SIZE), dtype=np.float32)
    for c in range(N_CORES):
        out[:, c * VS : (c + 1) * VS] = res.results[c]["vis_shT"].T
    kernel._last_result = res
    return out



# revision 3
# speedup vs baseline: 1.0275x; 1.0275x over previous
"""Trainium2 Bass kernel for the annealed mean-field Boltzmann machine.

Strategy: 1D tensor-parallel over 8 NeuronCores. Each core holds a
256-column shard of hh/vis_hid and a 512-column shard of vv/vis_hid.T,
all SBUF-resident in fp32 (the dynamics are chaotic in the early
annealing steps, so reduced-precision matmuls diverge; fp32 matches the
reference to the level of summation-order noise).

States are kept transposed (feature-on-partition, batch-on-free).
Every field matmul uses the weight tile as the stationary operand
(128x128) and a state k-tile (128x64) as the moving operand:
out[feat_tile, batch] += W[k, feat_tile].T @ stateT[k]. Outputs come
out feature-major, exactly the layout the next step needs, so there are
no transposes anywhere. Bias enters as a rank-1 matmul (bias x ones),
sigmoid/(1/temp) on the scalar engine, 0.9/0.1 mixing on the vector
engine, and each core's state shard is AllGathered so every core has
the full state for the next half-step.
"""

import sys
import time

sys.path.insert(0, "/opt/trn_rl_repo")

import numpy as np

N_CORES = 8
V_SIZE = 4096
H_SIZE = 2048
BATCH = 64
HS = H_SIZE // N_CORES  # 256 hid cols per core
VS = V_SIZE // N_CORES  # 512 vis cols per core
KT_H = H_SIZE // 128  # 16 k-tiles over hid features
KT_V = V_SIZE // 128  # 32 k-tiles over vis features
NTH = HS // 128  # 2 feature out-tiles per core (hid)
NTV = VS // 128  # 4 feature out-tiles per core (vis)

_BUILT = {}


def _build(n_steps: int, temps: np.ndarray, sim_mode: bool = False,
           no_comm: bool = False, splits=(3, (2, 6, 8), (1, 1, 6, 8))):
    import concourse.bacc as bacc
    import concourse.tile as tile
    import concourse.mybir as mybir

    F32 = mybir.dt.float32
    SIG = mybir.ActivationFunctionType.Sigmoid
    MULT = mybir.AluOpType.mult
    ADD = mybir.AluOpType.add

    nc = bacc.Bacc(
        "TRN2",
        target_bir_lowering=False,
        debug=False,
        enable_asserts=True,
        num_devices=1 if sim_mode else N_CORES,
    )

    def din(name, shape):
        return nc.dram_tensor(name, shape, F32, kind="ExternalInput").ap()

    xT = din("xT", [V_SIZE, BATCH])
    xT_my = din("xT_my", [VS, BATCH])
    hid0T = din("hid0T", [H_SIZE, BATCH])
    hh_w = din("hh_w", [H_SIZE, HS])
    vh_w = din("vh_w", [V_SIZE, HS])
    vv_w = din("vv_w", [V_SIZE, VS])
    vht_w = din("vht_w", [H_SIZE, VS])
    hb_row = din("hb_row", [1, HS])
    vb_row = din("vb_row", [1, VS])
    ones_row = din("ones_row", [1, BATCH])
    out_vis = nc.dram_tensor(
        "vis_shT", [VS, BATCH], F32, kind="ExternalOutput"
    ).ap()

    rg = [list(range(N_CORES))]
    shared_as = "Local" if sim_mode else "Shared"

    def all_gather(ag_in, ag_out, scratch):
        """Real AllGather, or in sim mode a 2-DMA pool-engine chain through a
        scratch DRAM tile: same ~5us latency and a single completion, without
        touching the engines/queues the real collective leaves free."""
        if not sim_mode:
            nc.gpsimd.collective_compute(
                "AllGather",
                mybir.AluOpType.bypass,
                replica_groups=rg,
                ins=[ag_in[:].opt()],
                outs=[ag_out[:].opt()],
            )
        else:
            rows = ag_in.shape[0]
            nc.gpsimd.dma_start(scratch[0:rows, :], ag_in[:])
            nc.gpsimd.dma_start(ag_out[:], scratch[:])

    with tile.TileContext(nc) as tc:
        with (
            tc.tile_pool(name="w", bufs=1) as wpool,
            tc.tile_pool(name="st", bufs=1) as stpool,
            tc.tile_pool(name="act", bufs=3) as actpool,
            tc.tile_pool(name="ps_h", bufs=2, space="PSUM") as ps_h,
            tc.tile_pool(name="ps_v", bufs=4, space="PSUM") as ps_v,
            tc.tile_pool(name="dram", bufs=2, space="DRAM") as dram,
        ):
            # --- constants + states first: tiny DMAs, so step 0's bias
            #     matmuls and first k-passes never queue behind 18 MiB of
            #     weights ---
            hb_sb = wpool.tile([1, HS], F32)
            vb_sb = wpool.tile([1, VS], F32)
            ones_sb = wpool.tile([1, BATCH], F32)
            nc.sync.dma_start(hb_sb[:], hb_row[:])
            nc.sync.dma_start(vb_sb[:], vb_row[:])
            nc.sync.dma_start(ones_sb[:], ones_row[:])

            visT = stpool.tile([128, KT_V, BATCH], F32)
            hidT = stpool.tile([128, KT_H, BATCH], F32)
            vmyT = stpool.tile([128, NTV, BATCH], F32)
            hmyT = stpool.tile([128, NTH, BATCH], F32)
            for j in range(0, KT_V, 8):
                nc.sync.dma_start(
                    visT[:, j : j + 8, :],
                    xT.rearrange("(k p) n -> p k n", p=128)[:, j : j + 8, :],
                )
            nc.sync.dma_start(vmyT[:], xT_my.rearrange("(k p) n -> p k n", p=128))
            nc.sync.dma_start(hidT[:], hid0T.rearrange("(k p) n -> p k n", p=128))
            nc.sync.dma_start(
                hmyT[:],
                hid0T.rearrange("(k p) n -> p k n", p=128)[:, :NTH, :],
            )

            # --- weights (SBUF-resident), blocked [k, j] 128x128, loaded in
            #     first-consumption order: hh, vh (hid field), vv, vht ---
            hh_sb = wpool.tile([128, KT_H, NTH, 128], F32)
            vh_sb = wpool.tile([128, KT_V, NTH, 128], F32)
            vv_sb = wpool.tile([128, KT_V, NTV, 128], F32)
            vht_sb = wpool.tile([128, KT_H, NTV, 128], F32)
            for j in range(0, KT_H, 4):
                nc.sync.dma_start(
                    hh_sb[:, j : j + 4, :, :],
                    hh_w.rearrange("(k p) (j n) -> p k j n", p=128, n=128)[
                        :, j : j + 4, :, :
                    ],
                )
            for j in range(0, KT_V, 4):
                nc.sync.dma_start(
                    vh_sb[:, j : j + 4, :, :],
                    vh_w.rearrange("(k p) (j n) -> p k j n", p=128, n=128)[
                        :, j : j + 4, :, :
                    ],
                )
            for j in range(0, KT_V, 4):
                nc.sync.dma_start(
                    vv_sb[:, j : j + 4, :, :],
                    vv_w.rearrange("(k p) (j n) -> p k j n", p=128, n=128)[
                        :, j : j + 4, :, :
                    ],
                )
            for j in range(0, KT_H, 4):
                nc.sync.dma_start(
                    vht_sb[:, j : j + 4, :, :],
                    vht_w.rearrange("(k p) (j n) -> p k j n", p=128, n=128)[
                        :, j : j + 4, :, :
                    ],
                )

            # vis and hid k-tiles are stored in AllGather output order
            # (host-side permutation), so restage is a contiguous copy and
            # consuming k in ascending order reads the early half first
            scr_h0 = dram.tile([H_SIZE, BATCH], F32, name="scr_h0", bufs=1)
            scr_h1 = dram.tile([H_SIZE // 2, BATCH], F32, name="scr_h1", bufs=1)
            scr_v0 = dram.tile([H_SIZE, BATCH], F32, name="scr_v0", bufs=1)
            scr_v1 = dram.tile([H_SIZE, BATCH], F32, name="scr_v1", bufs=1)
            scr_h = [scr_h0, scr_h1]
            scr_v = [scr_v0, scr_v1]
            KH2 = KT_H // 2  # 8: k-tiles per hid AllGather half
            KV2 = KT_V // 2  # 16: k-tiles per vis AllGather half
            # restage chunk ladder: small first chunk lands fast so the
            # first consumer matmuls start early
            RESTAGE_H = splits[1] if splits[1] else (2, 14)
            RESTAGE_V = splits[2] if splits[2] else (2, 14)

            for i in range(n_steps):
                inv_t = float(1.0 / temps[i])
                last = i == n_steps - 1

                # ---- field matmuls, ordered so that every comm chain is
                # covered by matmuls that do not depend on it ----
                phs = []
                for j in range(NTH):
                    ph = ps_h.tile(
                        [128, BATCH], F32, name=f"ph{i}_{j}", tag="ph"
                    )
                    phs.append(ph)
                    nc.tensor.matmul(
                        ph[:],
                        hb_sb[:, j * 128 : (j + 1) * 128],
                        ones_sb[:],
                        start=True,
                        stop=False,
                    )
                pvs = []
                for j in range(NTV):
                    pv = ps_v.tile(
                        [128, BATCH], F32, name=f"pv{i}_{j}", tag="pv"
                    )
                    pvs.append(pv)
                    nc.tensor.matmul(
                        pv[:],
                        vb_sb[:, j * 128 : (j + 1) * 128],
                        ones_sb[:],
                        start=True,
                        stop=False,
                    )
                # hid @ hh (needs hidT(i-1): available early)
                for k in range(KT_H):
                    for j in range(NTH):
                        nc.tensor.matmul(
                            phs[j][:], hh_sb[:, k, j, :], hidT[:, k, :],
                            start=False, stop=False,
                        )
                # vis-dependent matmuls, A-half k-tiles then B-half, with the
                # vv-A block between them as cover for the late AG_v_B
                for k in range(KV2):
                    for j in range(NTH):
                        nc.tensor.matmul(
                            phs[j][:], vh_sb[:, k, j, :], visT[:, k, :],
                            start=False, stop=False,
                        )
                VV_A = splits[0]
                for k in range(VV_A):
                    for j in range(NTV):
                        nc.tensor.matmul(
                            pvs[j][:], vv_sb[:, k, j, :], visT[:, k, :],
                            start=False, stop=False,
                        )
                for k in range(KV2, KT_V):
                    for j in range(NTH):
                        nc.tensor.matmul(
                            phs[j][:], vh_sb[:, k, j, :], visT[:, k, :],
                            start=False, stop=(k == KT_V - 1),
                        )
                # hid field complete: sigmoid + mix + AllGather
                for j in range(NTH):
                    ph = phs[j]
                    prob = actpool.tile(
                        [128, BATCH], F32, name=f"prh{i}_{j}", tag="pr"
                    )
                    nc.scalar.activation(prob[:], ph[:], SIG, scale=inv_t)
                    tmp = actpool.tile(
                        [128, BATCH], F32, name=f"tmh{i}_{j}", tag="tm"
                    )
                    nc.vector.tensor_sub(tmp[:], prob[:], hmyT[:, j, :])
                    nc.vector.scalar_tensor_tensor(
                        hmyT[:, j, :], tmp[:], 0.1, hmyT[:, j, :], MULT, ADD
                    )
                    if not no_comm:
                        if j == 0:
                            ag_in_h = dram.tile(
                                [HS, BATCH], F32, name=f"agih{i}", tag="agih"
                            )
                        nc.scalar.dma_start(
                            ag_in_h[j * 128 : (j + 1) * 128, :],
                            hmyT[:, j, :],
                        )
                if not no_comm:
                    ag_out_h = dram.tile(
                        [H_SIZE, BATCH], F32, addr_space=shared_as,
                        name=f"agoh{i}", tag="agoh",
                    )
                    all_gather(ag_in_h, ag_out_h, scr_h[0])
                    qs = 0
                    for w in RESTAGE_H:
                        nc.sync.dma_start(
                            hidT[:, qs : qs + w, :],
                            ag_out_h[:].rearrange("(k p) n -> p k n", p=128)[
                                :, qs : qs + w, :
                            ],
                        )
                        qs += w
                # finish vv for j0/j1 (covers the hid AllGather)
                for k in range(VV_A, KT_V):
                    for j in (0, 1):
                        nc.tensor.matmul(
                            pvs[j][:], vv_sb[:, k, j, :], visT[:, k, :],
                            start=False, stop=False,
                        )
                # hid(i)-dependent part of the vis field, then mix + AG per
                # j-pair; the j2/j3 vv tail runs between the two halves as
                # cover for AG_v_A
                for half in range(2):
                    js = (0, 1) if half == 0 else (2, 3)
                    for k in range(KT_H):
                        for j in js:
                            nc.tensor.matmul(
                                pvs[j][:], vht_sb[:, k, j, :], hidT[:, k, :],
                                start=False, stop=(k == KT_H - 1),
                            )
                    if half == 0:
                        for k in range(VV_A, KT_V):
                            for j in (2, 3):
                                nc.tensor.matmul(
                                    pvs[j][:], vv_sb[:, k, j, :], visT[:, k, :],
                                    start=False, stop=False,
                                )
                    for j in js:
                        pv = pvs[j]
                        prob = actpool.tile(
                            [128, BATCH], F32, name=f"prv{i}_{j}", tag="pr"
                        )
                        nc.scalar.activation(prob[:], pv[:], SIG, scale=inv_t)
                        tmp = actpool.tile(
                            [128, BATCH], F32, name=f"tmv{i}_{j}", tag="tm"
                        )
                        nc.vector.tensor_sub(tmp[:], prob[:], vmyT[:, j, :])
                        nc.vector.scalar_tensor_tensor(
                            vmyT[:, j, :], tmp[:], 0.1, vmyT[:, j, :], MULT, ADD
                        )
                        if not (last or no_comm):
                            if j % 2 == 0:
                                ag_in = dram.tile(
                                    [HS, BATCH], F32,
                                    name=f"agiv{i}_{half}", tag="agiv",
                                )
                            nc.scalar.dma_start(
                                ag_in[(j % 2) * 128 : (j % 2 + 1) * 128, :],
                                vmyT[:, j, :],
                            )
                    if last or no_comm:
                        continue
                    ag_out = dram.tile(
                        [H_SIZE, BATCH], F32, addr_space=shared_as,
                        name=f"agov{i}_{half}", tag="agov",
                    )
                    all_gather(ag_in, ag_out, scr_v[half])
                    qs = 0
                    for w in RESTAGE_V:
                        nc.sync.dma_start(
                            visT[:, KV2 * half + qs : KV2 * half + qs + w, :],
                            ag_out[:].rearrange("(k p) n -> p k n", p=128)[
                                :, qs : qs + w, :
                            ],
                        )
                        qs += w

            nc.sync.dma_start(
                out_vis[:].rearrange("(k p) n -> p k n", p=128), vmyT[:]
            )

    nc.compile()
    return nc


# vis k-tile permutation: SBUF order k' = AllGather output order.
# k' in [0,16): half A = each core's feature tiles {0,1};  orig k = 4c+t
# k' in [16,32): half B = tiles {2,3};                      orig k = 4c+2+t
_PERM_V = [4 * (k % 16 // 2) + (2 * (k // 16)) + (k % 2) for k in range(32)]
# hid k-tile permutation: per-j AllGather j=0 gathers each core's tile 0
# (orig 2c) into k' = c, j=1 gathers tile 1 (orig 2c+1) into k' = 8+c
_PERM_H = [2 * k for k in range(8)] + [2 * k + 1 for k in range(8)]


def _permute_vis_rows(a):
    """Reorder 128-row blocks of a (4096, ...) array into gather order."""
    blocks = a.reshape(32, 128, *a.shape[1:])
    return np.ascontiguousarray(blocks[_PERM_V].reshape(a.shape))


def _permute_hid_rows(a):
    """Reorder 128-row blocks of a (2048, ...) array into gather order."""
    blocks = a.reshape(16, 128, *a.shape[1:])
    return np.ascontiguousarray(blocks[_PERM_H].reshape(a.shape))


def _prep_inputs(x, vis_bias, hid_bias, vis_hid, vis_vis_raw, hid_hid_raw):
    f32 = np.float32
    vv = np.triu(np.asarray(vis_vis_raw, dtype=f32), 1)
    vv = vv + vv.T
    hh = np.triu(np.asarray(hid_hid_raw, dtype=f32), 1)
    hh = hh + hh.T
    vis_hid = np.ascontiguousarray(np.asarray(vis_hid, dtype=f32))
    vht = np.ascontiguousarray(vis_hid.T)  # (H, V)
    x = np.asarray(x, dtype=f32)
    xT = np.ascontiguousarray(x.T)
    ones = np.ones((1, BATCH), dtype=f32)
    hid0 = np.full((H_SIZE, BATCH), 0.5, dtype=f32)
    hb = np.ascontiguousarray(np.asarray(hid_bias, dtype=f32).reshape(1, H_SIZE))
    vb = np.ascontiguousarray(np.asarray(vis_bias, dtype=f32).reshape(1, V_SIZE))

    in_maps = []
    for c in range(N_CORES):
        hsl = slice(c * HS, (c + 1) * HS)
        vsl = slice(c * VS, (c + 1) * VS)
        in_maps.append(
            {
                "xT": _permute_vis_rows(xT),
                "xT_my": np.ascontiguousarray(xT[vsl]),
                "hid0T": hid0,
                "hh_w": np.ascontiguousarray(hh[:, hsl]),
                "vh_w": _permute_vis_rows(np.ascontiguousarray(vis_hid[:, hsl])),
                "vv_w": _permute_vis_rows(np.ascontiguousarray(vv[:, vsl])),
                "vht_w": np.ascontiguousarray(vht[:, vsl]),
                "hb_row": np.ascontiguousarray(hb[:, hsl]),
                "vb_row": np.ascontiguousarray(vb[:, vsl]),
                "ones_row": ones,
            }
        )
    return in_maps


def kernel(
    x,
    vis_bias,
    hid_bias,
    vis_hid,
    vis_vis_raw,
    hid_hid_raw,
    max_steps,
    _trace=False,
):
    from concourse import bass_utils

    n_steps = int(max_steps)
    steps_f = np.float32(n_steps)
    temps = (
        np.float32(0.01)
        * (
            np.float32(1.0)
            + np.float32(4.0)
            * np.exp(
                np.float32(-5.0)
                * np.arange(n_steps, dtype=np.float32)
                / steps_f
            )
        )
    ).astype(np.float32)

    if n_steps not in _BUILT:
        _BUILT[n_steps] = _build(n_steps, temps)
    nc = _BUILT[n_steps]

    in_maps = _prep_inputs(
        x, vis_bias, hid_bias, vis_hid, vis_vis_raw, hid_hid_raw
    )
    res = bass_utils.run_bass_kernel_spmd(
        nc, in_maps, core_ids=list(range(N_CORES)), trace=_trace
    )

    out = np.empty((BATCH, V_SIZE), dtype=np.float32)
    for c in range(N_CORES):
        out[:, c * VS : (c + 1) * VS] = res.results[c]["vis_shT"].T
    kernel._last_result = res
    return out



# revision 4
# speedup vs baseline: 1.0352x; 1.0075x over previous
"""Trainium2 Bass kernel for the annealed mean-field Boltzmann machine.

Strategy: 1D tensor-parallel over 8 NeuronCores. Each core holds a
256-column shard of hh/vis_hid and a 512-column shard of vv/vis_hid.T,
all SBUF-resident in fp32 (the dynamics are chaotic in the early
annealing steps, so reduced-precision matmuls diverge; fp32 matches the
reference to the level of summation-order noise).

States are kept transposed (feature-on-partition, batch-on-free).
Every field matmul uses the weight tile as the stationary operand
(128x128) and a state k-tile (128x64) as the moving operand:
out[feat_tile, batch] += W[k, feat_tile].T @ stateT[k]. Outputs come
out feature-major, exactly the layout the next step needs, so there are
no transposes anywhere. Bias enters as a rank-1 matmul (bias x ones),
sigmoid/(1/temp) on the scalar engine, 0.9/0.1 mixing on the vector
engine, and each core's state shard is AllGathered so every core has
the full state for the next half-step.
"""

import sys
import time

sys.path.insert(0, "/opt/trn_rl_repo")

import numpy as np

N_CORES = 8
V_SIZE = 4096
H_SIZE = 2048
BATCH = 64
HS = H_SIZE // N_CORES  # 256 hid cols per core
VS = V_SIZE // N_CORES  # 512 vis cols per core
KT_H = H_SIZE // 128  # 16 k-tiles over hid features
KT_V = V_SIZE // 128  # 32 k-tiles over vis features
NTH = HS // 128  # 2 feature out-tiles per core (hid)
NTV = VS // 128  # 4 feature out-tiles per core (vis)

_BUILT = {}


def _build(n_steps: int, temps: np.ndarray, sim_mode: bool = False,
           no_comm: bool = False, splits=(3, (2, 7, 7), (1, 1, 7, 7))):
    import concourse.bacc as bacc
    import concourse.tile as tile
    import concourse.mybir as mybir

    F32 = mybir.dt.float32
    SIG = mybir.ActivationFunctionType.Sigmoid
    MULT = mybir.AluOpType.mult
    ADD = mybir.AluOpType.add

    nc = bacc.Bacc(
        "TRN2",
        target_bir_lowering=False,
        debug=False,
        enable_asserts=True,
        num_devices=1 if sim_mode else N_CORES,
    )

    def din(name, shape):
        return nc.dram_tensor(name, shape, F32, kind="ExternalInput").ap()

    xT = din("xT", [V_SIZE, BATCH])
    xT_my = din("xT_my", [VS, BATCH])
    hid0T = din("hid0T", [H_SIZE, BATCH])
    hh_w = din("hh_w", [H_SIZE, HS])
    vh_w = din("vh_w", [V_SIZE, HS])
    vv_w = din("vv_w", [V_SIZE, VS])
    vht_w = din("vht_w", [H_SIZE, VS])
    hb_row = din("hb_row", [1, HS])
    vb_row = din("vb_row", [1, VS])
    ones_row = din("ones_row", [1, BATCH])
    out_vis = nc.dram_tensor(
        "vis_shT", [VS, BATCH], F32, kind="ExternalOutput"
    ).ap()

    rg = [list(range(N_CORES))]
    shared_as = "Local" if sim_mode else "Shared"

    def all_gather(ag_in, ag_out, scratch):
        """Real AllGather, or in sim mode a 2-DMA pool-engine chain through a
        scratch DRAM tile: same ~5us latency and a single completion, without
        touching the engines/queues the real collective leaves free."""
        if not sim_mode:
            nc.gpsimd.collective_compute(
                "AllGather",
                mybir.AluOpType.bypass,
                replica_groups=rg,
                ins=[ag_in[:].opt()],
                outs=[ag_out[:].opt()],
            )
        else:
            rows = ag_in.shape[0]
            nc.gpsimd.dma_start(scratch[0:rows, :], ag_in[:])
            nc.gpsimd.dma_start(ag_out[:], scratch[:])

    with tile.TileContext(nc) as tc:
        with (
            tc.tile_pool(name="w", bufs=1) as wpool,
            tc.tile_pool(name="st", bufs=1) as stpool,
            tc.tile_pool(name="act", bufs=3) as actpool,
            tc.tile_pool(name="ps_h", bufs=2, space="PSUM") as ps_h,
            tc.tile_pool(name="ps_v", bufs=4, space="PSUM") as ps_v,
            tc.tile_pool(name="dram", bufs=2, space="DRAM") as dram,
        ):
            # --- constants + states first: tiny DMAs, so step 0's bias
            #     matmuls and first k-passes never queue behind 18 MiB of
            #     weights ---
            hb_sb = wpool.tile([1, HS], F32)
            vb_sb = wpool.tile([1, VS], F32)
            ones_sb = wpool.tile([1, BATCH], F32)
            nc.sync.dma_start(hb_sb[:], hb_row[:])
            nc.sync.dma_start(vb_sb[:], vb_row[:])
            nc.sync.dma_start(ones_sb[:], ones_row[:])

            visT = stpool.tile([128, KT_V, BATCH], F32)
            hidT = stpool.tile([128, KT_H, BATCH], F32)
            vmyT = stpool.tile([128, NTV, BATCH], F32)
            hmyT = stpool.tile([128, NTH, BATCH], F32)
            for j in range(0, KT_V, 8):
                nc.sync.dma_start(
                    visT[:, j : j + 8, :],
                    xT.rearrange("(k p) n -> p k n", p=128)[:, j : j + 8, :],
                )
            nc.sync.dma_start(vmyT[:], xT_my.rearrange("(k p) n -> p k n", p=128))
            nc.sync.dma_start(hidT[:], hid0T.rearrange("(k p) n -> p k n", p=128))
            nc.sync.dma_start(
                hmyT[:],
                hid0T.rearrange("(k p) n -> p k n", p=128)[:, :NTH, :],
            )

            # --- weights (SBUF-resident), blocked [k, j] 128x128, loaded in
            #     first-consumption order: hh, vh (hid field), vv, vht ---
            hh_sb = wpool.tile([128, KT_H, NTH, 128], F32)
            vh_sb = wpool.tile([128, KT_V, NTH, 128], F32)
            vv_sb = wpool.tile([128, KT_V, NTV, 128], F32)
            vht_sb = wpool.tile([128, KT_H, NTV, 128], F32)
            for j in range(0, KT_H, 4):
                nc.sync.dma_start(
                    hh_sb[:, j : j + 4, :, :],
                    hh_w.rearrange("(k p) (j n) -> p k j n", p=128, n=128)[
                        :, j : j + 4, :, :
                    ],
                )
            for j in range(0, KT_V, 4):
                nc.sync.dma_start(
                    vh_sb[:, j : j + 4, :, :],
                    vh_w.rearrange("(k p) (j n) -> p k j n", p=128, n=128)[
                        :, j : j + 4, :, :
                    ],
                )
            for j in range(0, KT_V, 4):
                nc.sync.dma_start(
                    vv_sb[:, j : j + 4, :, :],
                    vv_w.rearrange("(k p) (j n) -> p k j n", p=128, n=128)[
                        :, j : j + 4, :, :
                    ],
                )
            for j in range(0, KT_H, 4):
                nc.sync.dma_start(
                    vht_sb[:, j : j + 4, :, :],
                    vht_w.rearrange("(k p) (j n) -> p k j n", p=128, n=128)[
                        :, j : j + 4, :, :
                    ],
                )

            # vis and hid k-tiles are stored in AllGather output order
            # (host-side permutation), so restage is a contiguous copy and
            # consuming k in ascending order reads the early half first
            scr_h0 = dram.tile([H_SIZE, BATCH], F32, name="scr_h0", bufs=1)
            scr_h1 = dram.tile([H_SIZE // 2, BATCH], F32, name="scr_h1", bufs=1)
            scr_v0 = dram.tile([H_SIZE, BATCH], F32, name="scr_v0", bufs=1)
            scr_v1 = dram.tile([H_SIZE, BATCH], F32, name="scr_v1", bufs=1)
            scr_h = [scr_h0, scr_h1]
            scr_v = [scr_v0, scr_v1]
            KH2 = KT_H // 2  # 8: k-tiles per hid AllGather half
            KV2 = KT_V // 2  # 16: k-tiles per vis AllGather half
            # restage chunk ladder: small first chunk lands fast so the
            # first consumer matmuls start early
            RESTAGE_H = splits[1] if splits[1] else (2, 14)
            RESTAGE_V = splits[2] if splits[2] else (2, 14)

            for i in range(n_steps):
                inv_t = float(1.0 / temps[i])
                last = i == n_steps - 1

                # ---- field matmuls, ordered so that every comm chain is
                # covered by matmuls that do not depend on it ----
                phs = []
                for j in range(NTH):
                    ph = ps_h.tile(
                        [128, BATCH], F32, name=f"ph{i}_{j}", tag="ph"
                    )
                    phs.append(ph)
                    nc.tensor.matmul(
                        ph[:],
                        hb_sb[:, j * 128 : (j + 1) * 128],
                        ones_sb[:],
                        start=True,
                        stop=False,
                    )
                pvs = []
                for j in range(NTV):
                    pv = ps_v.tile(
                        [128, BATCH], F32, name=f"pv{i}_{j}", tag="pv"
                    )
                    pvs.append(pv)
                    nc.tensor.matmul(
                        pv[:],
                        vb_sb[:, j * 128 : (j + 1) * 128],
                        ones_sb[:],
                        start=True,
                        stop=False,
                    )
                # hid @ hh (needs hidT(i-1): available early)
                for k in range(KT_H):
                    for j in range(NTH):
                        nc.tensor.matmul(
                            phs[j][:], hh_sb[:, k, j, :], hidT[:, k, :],
                            start=False, stop=False,
                        )
                # vis-dependent matmuls, A-half k-tiles then B-half, with the
                # vv-A block between them as cover for the late AG_v_B
                for k in range(KV2):
                    for j in range(NTH):
                        nc.tensor.matmul(
                            phs[j][:], vh_sb[:, k, j, :], visT[:, k, :],
                            start=False, stop=False,
                        )
                VV_A = splits[0]
                for k in range(VV_A):
                    for j in range(NTV):
                        nc.tensor.matmul(
                            pvs[j][:], vv_sb[:, k, j, :], visT[:, k, :],
                            start=False, stop=False,
                        )
                for k in range(KV2, KT_V):
                    for j in range(NTH):
                        nc.tensor.matmul(
                            phs[j][:], vh_sb[:, k, j, :], visT[:, k, :],
                            start=False, stop=(k == KT_V - 1),
                        )
                # hid field complete: sigmoid + mix + AllGather
                for j in range(NTH):
                    ph = phs[j]
                    prob = actpool.tile(
                        [128, BATCH], F32, name=f"prh{i}_{j}", tag="pr"
                    )
                    nc.scalar.activation(prob[:], ph[:], SIG, scale=inv_t)
                    tmp = actpool.tile(
                        [128, BATCH], F32, name=f"tmh{i}_{j}", tag="tm"
                    )
                    nc.vector.tensor_sub(tmp[:], prob[:], hmyT[:, j, :])
                    nc.vector.scalar_tensor_tensor(
                        hmyT[:, j, :], tmp[:], 0.1, hmyT[:, j, :], MULT, ADD
                    )
                    if not no_comm:
                        if j == 0:
                            ag_in_h = dram.tile(
                                [HS, BATCH], F32, name=f"agih{i}", tag="agih"
                            )
                        nc.scalar.dma_start(
                            ag_in_h[j * 128 : (j + 1) * 128, :],
                            hmyT[:, j, :],
                        )
                if not no_comm:
                    ag_out_h = dram.tile(
                        [H_SIZE, BATCH], F32, addr_space=shared_as,
                        name=f"agoh{i}", tag="agoh",
                    )
                    all_gather(ag_in_h, ag_out_h, scr_h[0])
                    qs = 0
                    for w in RESTAGE_H:
                        nc.sync.dma_start(
                            hidT[:, qs : qs + w, :],
                            ag_out_h[:].rearrange("(k p) n -> p k n", p=128)[
                                :, qs : qs + w, :
                            ],
                        )
                        qs += w
                # finish vv for j0/j1 (covers the hid AllGather)
                for k in range(VV_A, KT_V):
                    for j in (0, 1):
                        nc.tensor.matmul(
                            pvs[j][:], vv_sb[:, k, j, :], visT[:, k, :],
                            start=False, stop=False,
                        )
                # hid(i)-dependent part of the vis field, then mix + AG per
                # j-pair; the j2/j3 vv tail runs between the two halves as
                # cover for AG_v_A
                for half in range(2):
                    js = (0, 1) if half == 0 else (2, 3)
                    for k in range(KT_H):
                        for j in js:
                            nc.tensor.matmul(
                                pvs[j][:], vht_sb[:, k, j, :], hidT[:, k, :],
                                start=False, stop=(k == KT_H - 1),
                            )
                    if half == 0:
                        for k in range(VV_A, KT_V):
                            for j in (2, 3):
                                nc.tensor.matmul(
                                    pvs[j][:], vv_sb[:, k, j, :], visT[:, k, :],
                                    start=False, stop=False,
                                )
                    for j in js:
                        pv = pvs[j]
                        prob = actpool.tile(
                            [128, BATCH], F32, name=f"prv{i}_{j}", tag="pr"
                        )
                        nc.scalar.activation(prob[:], pv[:], SIG, scale=inv_t)
                        tmp = actpool.tile(
                            [128, BATCH], F32, name=f"tmv{i}_{j}", tag="tm"
                        )
                        nc.vector.tensor_sub(tmp[:], prob[:], vmyT[:, j, :])
                        nc.vector.scalar_tensor_tensor(
                            vmyT[:, j, :], tmp[:], 0.1, vmyT[:, j, :], MULT, ADD
                        )
                        if not (last or no_comm):
                            if j % 2 == 0:
                                ag_in = dram.tile(
                                    [HS, BATCH], F32,
                                    name=f"agiv{i}_{half}", tag="agiv",
                                )
                            nc.scalar.dma_start(
                                ag_in[(j % 2) * 128 : (j % 2 + 1) * 128, :],
                                vmyT[:, j, :],
                            )
                    if last or no_comm:
                        continue
                    ag_out = dram.tile(
                        [H_SIZE, BATCH], F32, addr_space=shared_as,
                        name=f"agov{i}_{half}", tag="agov",
                    )
                    all_gather(ag_in, ag_out, scr_v[half])
                    qs = 0
                    for w in RESTAGE_V:
                        nc.sync.dma_start(
                            visT[:, KV2 * half + qs : KV2 * half + qs + w, :],
                            ag_out[:].rearrange("(k p) n -> p k n", p=128)[
                                :, qs : qs + w, :
                            ],
                        )
                        qs += w

            nc.sync.dma_start(
                out_vis[:].rearrange("(k p) n -> p k n", p=128), vmyT[:]
            )

    nc.compile()
    return nc


# vis k-tile permutation: SBUF order k' = AllGather output order.
# k' in [0,16): half A = each core's feature tiles {0,1};  orig k = 4c+t
# k' in [16,32): half B = tiles {2,3};                      orig k = 4c+2+t
_PERM_V = [4 * (k % 16 // 2) + (2 * (k // 16)) + (k % 2) for k in range(32)]
# hid k-tile permutation: per-j AllGather j=0 gathers each core's tile 0
# (orig 2c) into k' = c, j=1 gathers tile 1 (orig 2c+1) into k' = 8+c
_PERM_H = [2 * k for k in range(8)] + [2 * k + 1 for k in range(8)]


def _permute_vis_rows(a):
    """Reorder 128-row blocks of a (4096, ...) array into gather order."""
    blocks = a.reshape(32, 128, *a.shape[1:])
    return np.ascontiguousarray(blocks[_PERM_V].reshape(a.shape))


def _permute_hid_rows(a):
    """Reorder 128-row blocks of a (2048, ...) array into gather order."""
    blocks = a.reshape(16, 128, *a.shape[1:])
    return np.ascontiguousarray(blocks[_PERM_H].reshape(a.shape))


def _prep_inputs(x, vis_bias, hid_bias, vis_hid, vis_vis_raw, hid_hid_raw):
    f32 = np.float32
    vv = np.triu(np.asarray(vis_vis_raw, dtype=f32), 1)
    vv = vv + vv.T
    hh = np.triu(np.asarray(hid_hid_raw, dtype=f32), 1)
    hh = hh + hh.T
    vis_hid = np.ascontiguousarray(np.asarray(vis_hid, dtype=f32))
    vht = np.ascontiguousarray(vis_hid.T)  # (H, V)
    x = np.asarray(x, dtype=f32)
    xT = np.ascontiguousarray(x.T)
    ones = np.ones((1, BATCH), dtype=f32)
    hid0 = np.full((H_SIZE, BATCH), 0.5, dtype=f32)
    hb = np.ascontiguousarray(np.asarray(hid_bias, dtype=f32).reshape(1, H_SIZE))
    vb = np.ascontiguousarray(np.asarray(vis_bias, dtype=f32).reshape(1, V_SIZE))

    in_maps = []
    for c in range(N_CORES):
        hsl = slice(c * HS, (c + 1) * HS)
        vsl = slice(c * VS, (c + 1) * VS)
        in_maps.append(
            {
                "xT": _permute_vis_rows(xT),
                "xT_my": np.ascontiguousarray(xT[vsl]),
                "hid0T": hid0,
                "hh_w": np.ascontiguousarray(hh[:, hsl]),
                "vh_w": _permute_vis_rows(np.ascontiguousarray(vis_hid[:, hsl])),
                "vv_w": _permute_vis_rows(np.ascontiguousarray(vv[:, vsl])),
                "vht_w": np.ascontiguousarray(vht[:, vsl]),
                "hb_row": np.ascontiguousarray(hb[:, hsl]),
                "vb_row": np.ascontiguousarray(vb[:, vsl]),
                "ones_row": ones,
            }
        )
    return in_maps


def kernel(
    x,
    vis_bias,
    hid_bias,
    vis_hid,
    vis_vis_raw,
    hid_hid_raw,
    max_steps,
    _trace=False,
):
    from concourse import bass_utils

    n_steps = int(max_steps)
    steps_f = np.float32(n_steps)
    temps = (
        np.float32(0.01)
        * (
            np.float32(1.0)
            + np.float32(4.0)
            * np.exp(
                np.float32(-5.0)
                * np.arange(n_steps, dtype=np.float32)
                / steps_f
            )
        )
    ).astype(np.float32)

    if n_steps not in _BUILT:
        _BUILT[n_steps] = _build(n_steps, temps)
    nc = _BUILT[n_steps]

    in_maps = _prep_inputs(
        x, vis_bias, hid_bias, vis_hid, vis_vis_raw, hid_hid_raw
    )
    res = bass_utils.run_bass_kernel_spmd(
        nc, in_maps, core_ids=list(range(N_CORES)), trace=_trace
    )

    out = np.empty((BATCH, V_SIZE), dtype=np.float32)
    for c in range(N_CORES):
        out[:, c * VS : (c + 1) * VS] = res.results[c]["vis_shT"].T
    kernel._last_result = res
    return out



# revision 6
# speedup vs baseline: 1.0477x; 1.0121x over previous
"""Trainium2 Bass kernel for the annealed mean-field Boltzmann machine.

Strategy: 1D tensor-parallel over 8 NeuronCores. Each core holds a
256-column shard of hh/vis_hid and a 512-column shard of vv/vis_hid.T,
all SBUF-resident in fp32 (the dynamics are chaotic in the early
annealing steps, so reduced-precision matmuls diverge; fp32 matches the
reference to the level of summation-order noise).

States are kept transposed (feature-on-partition, batch-on-free).
Every field matmul uses the weight tile as the stationary operand
(128x128) and a state k-tile (128x64) as the moving operand:
out[feat_tile, batch] += W[k, feat_tile].T @ stateT[k]. Outputs come
out feature-major, exactly the layout the next step needs, so there are
no transposes anywhere. Bias enters as a rank-1 matmul (bias x ones),
sigmoid/(1/temp) on the scalar engine, 0.9/0.1 mixing on the vector
engine, and each core's state shard is AllGathered so every core has
the full state for the next half-step.
"""

import sys
import time

sys.path.insert(0, "/opt/trn_rl_repo")

import numpy as np

N_CORES = 8
V_SIZE = 4096
H_SIZE = 2048
BATCH = 64
HS = H_SIZE // N_CORES  # 256 hid cols per core
VS = V_SIZE // N_CORES  # 512 vis cols per core
KT_H = H_SIZE // 128  # 16 k-tiles over hid features
KT_V = V_SIZE // 128  # 32 k-tiles over vis features
NTH = HS // 128  # 2 feature out-tiles per core (hid)
NTV = VS // 128  # 4 feature out-tiles per core (vis)

_BUILT = {}


def _build(n_steps: int, temps: np.ndarray, sim_mode: bool = False,
           no_comm: bool = False, splits=(3, (2, 7, 7), (1, 1, 7, 7), (3, 6, 7))):
    import concourse.bacc as bacc
    import concourse.tile as tile
    import concourse.mybir as mybir

    F32 = mybir.dt.float32
    SIG = mybir.ActivationFunctionType.Sigmoid
    MULT = mybir.AluOpType.mult
    ADD = mybir.AluOpType.add

    nc = bacc.Bacc(
        "TRN2",
        target_bir_lowering=False,
        debug=False,
        enable_asserts=True,
        num_devices=1 if sim_mode else N_CORES,
    )

    def din(name, shape):
        return nc.dram_tensor(name, shape, F32, kind="ExternalInput").ap()

    xT = din("xT", [V_SIZE, BATCH])
    xT_my = din("xT_my", [VS, BATCH])
    hid0T = din("hid0T", [H_SIZE, BATCH])
    hh_w = din("hh_w", [H_SIZE, HS])
    vh_w = din("vh_w", [V_SIZE, HS])
    vv_w = din("vv_w", [V_SIZE, VS])
    vht_w = din("vht_w", [H_SIZE, VS])
    hb_row = din("hb_row", [1, HS])
    vb_row = din("vb_row", [1, VS])
    ones_row = din("ones_row", [1, BATCH])
    out_vis = nc.dram_tensor(
        "vis_shT", [VS, BATCH], F32, kind="ExternalOutput"
    ).ap()

    rg = [list(range(N_CORES))]
    shared_as = "Local" if sim_mode else "Shared"

    def all_gather(ag_in, ag_out, scratch):
        """Real AllGather, or in sim mode a 2-DMA pool-engine chain through a
        scratch DRAM tile: same ~5us latency and a single completion, without
        touching the engines/queues the real collective leaves free."""
        if not sim_mode:
            nc.gpsimd.collective_compute(
                "AllGather",
                mybir.AluOpType.bypass,
                replica_groups=rg,
                ins=[ag_in[:].opt()],
                outs=[ag_out[:].opt()],
            )
        else:
            rows = ag_in.shape[0]
            nc.gpsimd.dma_start(scratch[0:rows, :], ag_in[:])
            nc.gpsimd.dma_start(ag_out[:], scratch[:])

    with tile.TileContext(nc) as tc:
        with (
            tc.tile_pool(name="w", bufs=1) as wpool,
            tc.tile_pool(name="st", bufs=1) as stpool,
            tc.tile_pool(name="act", bufs=3) as actpool,
            tc.tile_pool(name="ps_h", bufs=2, space="PSUM") as ps_h,
            tc.tile_pool(name="ps_v", bufs=4, space="PSUM") as ps_v,
            tc.tile_pool(name="dram", bufs=2, space="DRAM") as dram,
        ):
            # --- constants + states first: tiny DMAs, so step 0's bias
            #     matmuls and first k-passes never queue behind 18 MiB of
            #     weights ---
            hb_sb = wpool.tile([1, HS], F32)
            vb_sb = wpool.tile([1, VS], F32)
            ones_sb = wpool.tile([1, BATCH], F32)
            nc.sync.dma_start(hb_sb[:], hb_row[:])
            nc.sync.dma_start(vb_sb[:], vb_row[:])
            nc.sync.dma_start(ones_sb[:], ones_row[:])

            visT = stpool.tile([128, KT_V, BATCH], F32)
            hidT = stpool.tile([128, KT_H, BATCH], F32)
            vmyT = stpool.tile([128, NTV, BATCH], F32)
            hmyT = stpool.tile([128, NTH, BATCH], F32)
            for j in range(0, KT_V, 8):
                nc.sync.dma_start(
                    visT[:, j : j + 8, :],
                    xT.rearrange("(k p) n -> p k n", p=128)[:, j : j + 8, :],
                )
            nc.sync.dma_start(vmyT[:], xT_my.rearrange("(k p) n -> p k n", p=128))
            nc.sync.dma_start(hidT[:], hid0T.rearrange("(k p) n -> p k n", p=128))
            nc.sync.dma_start(
                hmyT[:],
                hid0T.rearrange("(k p) n -> p k n", p=128)[:, :NTH, :],
            )

            # --- weights (SBUF-resident), blocked [k, j] 128x128, loaded in
            #     first-consumption order: hh, vh (hid field), vv, vht ---
            hh_sb = wpool.tile([128, KT_H, NTH, 128], F32)
            vh_sb = wpool.tile([128, KT_V, NTH, 128], F32)
            vv_sb = wpool.tile([128, KT_V, NTV, 128], F32)
            vht_sb = wpool.tile([128, KT_H, NTV, 128], F32)
            for j in range(0, KT_H, 4):
                nc.sync.dma_start(
                    hh_sb[:, j : j + 4, :, :],
                    hh_w.rearrange("(k p) (j n) -> p k j n", p=128, n=128)[
                        :, j : j + 4, :, :
                    ],
                )
            for j in range(0, KT_V, 4):
                nc.sync.dma_start(
                    vh_sb[:, j : j + 4, :, :],
                    vh_w.rearrange("(k p) (j n) -> p k j n", p=128, n=128)[
                        :, j : j + 4, :, :
                    ],
                )
            for j in range(0, KT_V, 4):
                nc.sync.dma_start(
                    vv_sb[:, j : j + 4, :, :],
                    vv_w.rearrange("(k p) (j n) -> p k j n", p=128, n=128)[
                        :, j : j + 4, :, :
                    ],
                )
            for j in range(0, KT_H, 4):
                nc.sync.dma_start(
                    vht_sb[:, j : j + 4, :, :],
                    vht_w.rearrange("(k p) (j n) -> p k j n", p=128, n=128)[
                        :, j : j + 4, :, :
                    ],
                )

            # vis and hid k-tiles are stored in AllGather output order
            # (host-side permutation), so restage is a contiguous copy and
            # consuming k in ascending order reads the early half first
            scr_h0 = dram.tile([H_SIZE, BATCH], F32, name="scr_h0", bufs=1)
            scr_h1 = dram.tile([H_SIZE // 2, BATCH], F32, name="scr_h1", bufs=1)
            scr_v0 = dram.tile([H_SIZE, BATCH], F32, name="scr_v0", bufs=1)
            scr_v1 = dram.tile([H_SIZE, BATCH], F32, name="scr_v1", bufs=1)
            scr_h = [scr_h0, scr_h1]
            scr_v = [scr_v0, scr_v1]
            KH2 = KT_H // 2  # 8: k-tiles per hid AllGather half
            KV2 = KT_V // 2  # 16: k-tiles per vis AllGather half
            # restage chunk ladder: small first chunk lands fast so the
            # first consumer matmuls start early
            RESTAGE_H = splits[1] if splits[1] else (2, 14)
            RESTAGE_V = splits[2] if splits[2] else (2, 14)
            RESTAGE_VB = splits[3] if len(splits) > 3 else RESTAGE_V

            for i in range(n_steps):
                inv_t = float(1.0 / temps[i])
                last = i == n_steps - 1

                # ---- field matmuls, ordered so that every comm chain is
                # covered by matmuls that do not depend on it ----
                phs = []
                for j in range(NTH):
                    ph = ps_h.tile(
                        [128, BATCH], F32, name=f"ph{i}_{j}", tag="ph"
                    )
                    phs.append(ph)
                    nc.tensor.matmul(
                        ph[:],
                        hb_sb[:, j * 128 : (j + 1) * 128],
                        ones_sb[:],
                        start=True,
                        stop=False,
                    )
                pvs = []
                for j in range(NTV):
                    pv = ps_v.tile(
                        [128, BATCH], F32, name=f"pv{i}_{j}", tag="pv"
                    )
                    pvs.append(pv)
                    nc.tensor.matmul(
                        pv[:],
                        vb_sb[:, j * 128 : (j + 1) * 128],
                        ones_sb[:],
                        start=True,
                        stop=False,
                    )
                # hid @ hh (needs hidT(i-1): available early)
                for k in range(KT_H):
                    for j in range(NTH):
                        nc.tensor.matmul(
                            phs[j][:], hh_sb[:, k, j, :], hidT[:, k, :],
                            start=False, stop=False,
                        )
                # vis-dependent matmuls, A-half k-tiles then B-half, with the
                # vv-A block between them as cover for the late AG_v_B
                for k in range(KV2):
                    for j in range(NTH):
                        nc.tensor.matmul(
                            phs[j][:], vh_sb[:, k, j, :], visT[:, k, :],
                            start=False, stop=False,
                        )
                VV_A = splits[0]
                for k in range(VV_A):
                    for j in range(NTV):
                        nc.tensor.matmul(
                            pvs[j][:], vv_sb[:, k, j, :], visT[:, k, :],
                            start=False, stop=False,
                        )
                for k in range(KV2, KT_V):
                    for j in range(NTH):
                        nc.tensor.matmul(
                            phs[j][:], vh_sb[:, k, j, :], visT[:, k, :],
                            start=False, stop=(k == KT_V - 1),
                        )
                # hid field complete: sigmoid + mix + AllGather
                for j in range(NTH):
                    ph = phs[j]
                    prob = actpool.tile(
                        [128, BATCH], F32, name=f"prh{i}_{j}", tag="pr"
                    )
                    nc.scalar.activation(prob[:], ph[:], SIG, scale=inv_t)
                    tmp = actpool.tile(
                        [128, BATCH], F32, name=f"tmh{i}_{j}", tag="tm"
                    )
                    nc.vector.tensor_sub(tmp[:], prob[:], hmyT[:, j, :])
                    nc.vector.scalar_tensor_tensor(
                        hmyT[:, j, :], tmp[:], 0.1, hmyT[:, j, :], MULT, ADD
                    )
                    if not no_comm:
                        if j == 0:
                            ag_in_h = dram.tile(
                                [HS, BATCH], F32, name=f"agih{i}", tag="agih"
                            )
                        nc.scalar.dma_start(
                            ag_in_h[j * 128 : (j + 1) * 128, :],
                            hmyT[:, j, :],
                        )
                if not no_comm:
                    ag_out_h = dram.tile(
                        [H_SIZE, BATCH], F32, addr_space=shared_as,
                        name=f"agoh{i}", tag="agoh",
                    )
                    all_gather(ag_in_h, ag_out_h, scr_h[0])
                    qs = 0
                    for w in RESTAGE_H:
                        nc.sync.dma_start(
                            hidT[:, qs : qs + w, :],
                            ag_out_h[:].rearrange("(k p) n -> p k n", p=128)[
                                :, qs : qs + w, :
                            ],
                        )
                        qs += w
                # finish vv for j0/j1 (covers the hid AllGather)
                for k in range(VV_A, KT_V):
                    for j in (0, 1):
                        nc.tensor.matmul(
                            pvs[j][:], vv_sb[:, k, j, :], visT[:, k, :],
                            start=False, stop=False,
                        )
                # hid(i)-dependent part of the vis field, then mix + AG per
                # j-pair; the j2/j3 vv tail runs between the two halves as
                # cover for AG_v_A
                for half in range(2):
                    js = (0, 1) if half == 0 else (2, 3)
                    for k in range(KT_H):
                        for j in js:
                            nc.tensor.matmul(
                                pvs[j][:], vht_sb[:, k, j, :], hidT[:, k, :],
                                start=False, stop=(k == KT_H - 1),
                            )
                    if half == 0:
                        for k in range(VV_A, KT_V):
                            for j in (2, 3):
                                nc.tensor.matmul(
                                    pvs[j][:], vv_sb[:, k, j, :], visT[:, k, :],
                                    start=False, stop=False,
                                )
                    for j in js:
                        pv = pvs[j]
                        prob = actpool.tile(
                            [128, BATCH], F32, name=f"prv{i}_{j}", tag="pr"
                        )
                        nc.scalar.activation(prob[:], pv[:], SIG, scale=inv_t)
                        tmp = actpool.tile(
                            [128, BATCH], F32, name=f"tmv{i}_{j}", tag="tm"
                        )
                        nc.vector.tensor_sub(tmp[:], prob[:], vmyT[:, j, :])
                        nc.vector.scalar_tensor_tensor(
                            vmyT[:, j, :], tmp[:], 0.1, vmyT[:, j, :], MULT, ADD
                        )
                        if not (last or no_comm):
                            if j % 2 == 0:
                                ag_in = dram.tile(
                                    [HS, BATCH], F32,
                                    name=f"agiv{i}_{half}", tag="agiv",
                                )
                            nc.scalar.dma_start(
                                ag_in[(j % 2) * 128 : (j % 2 + 1) * 128, :],
                                vmyT[:, j, :],
                            )
                    if last or no_comm:
                        continue
                    ag_out = dram.tile(
                        [H_SIZE, BATCH], F32, addr_space=shared_as,
                        name=f"agov{i}_{half}", tag="agov",
                    )
                    all_gather(ag_in, ag_out, scr_v[half])
                    qs = 0
                    for w in (RESTAGE_V if half == 0 else RESTAGE_VB):
                        nc.sync.dma_start(
                            visT[:, KV2 * half + qs : KV2 * half + qs + w, :],
                            ag_out[:].rearrange("(k p) n -> p k n", p=128)[
                                :, qs : qs + w, :
                            ],
                        )
                        qs += w

            nc.sync.dma_start(
                out_vis[:].rearrange("(k p) n -> p k n", p=128), vmyT[:]
            )

    nc.compile()
    return nc


# vis k-tile permutation: SBUF order k' = AllGather output order.
# k' in [0,16): half A = each core's feature tiles {0,1};  orig k = 4c+t
# k' in [16,32): half B = tiles {2,3};                      orig k = 4c+2+t
_PERM_V = [4 * (k % 16 // 2) + (2 * (k // 16)) + (k % 2) for k in range(32)]
# hid k-tile permutation: per-j AllGather j=0 gathers each core's tile 0
# (orig 2c) into k' = c, j=1 gathers tile 1 (orig 2c+1) into k' = 8+c
_PERM_H = [2 * k for k in range(8)] + [2 * k + 1 for k in range(8)]


def _permute_vis_rows(a):
    """Reorder 128-row blocks of a (4096, ...) array into gather order."""
    blocks = a.reshape(32, 128, *a.shape[1:])
    return np.ascontiguousarray(blocks[_PERM_V].reshape(a.shape))


def _permute_hid_rows(a):
    """Reorder 128-row blocks of a (2048, ...) array into gather order."""
    blocks = a.reshape(16, 128, *a.shape[1:])
    return np.ascontiguousarray(blocks[_PERM_H].reshape(a.shape))


def _prep_inputs(x, vis_bias, hid_bias, vis_hid, vis_vis_raw, hid_hid_raw):
    f32 = np.float32
    vv = np.triu(np.asarray(vis_vis_raw, dtype=f32), 1)
    vv = vv + vv.T
    hh = np.triu(np.asarray(hid_hid_raw, dtype=f32), 1)
    hh = hh + hh.T
    vis_hid = np.ascontiguousarray(np.asarray(vis_hid, dtype=f32))
    vht = np.ascontiguousarray(vis_hid.T)  # (H, V)
    x = np.asarray(x, dtype=f32)
    xT = np.ascontiguousarray(x.T)
    ones = np.ones((1, BATCH), dtype=f32)
    hid0 = np.full((H_SIZE, BATCH), 0.5, dtype=f32)
    hb = np.ascontiguousarray(np.asarray(hid_bias, dtype=f32).reshape(1, H_SIZE))
    vb = np.ascontiguousarray(np.asarray(vis_bias, dtype=f32).reshape(1, V_SIZE))

    in_maps = []
    for c in range(N_CORES):
        hsl = slice(c * HS, (c + 1) * HS)
        vsl = slice(c * VS, (c + 1) * VS)
        in_maps.append(
            {
                "xT": _permute_vis_rows(xT),
                "xT_my": np.ascontiguousarray(xT[vsl]),
                "hid0T": hid0,
                "hh_w": np.ascontiguousarray(hh[:, hsl]),
                "vh_w": _permute_vis_rows(np.ascontiguousarray(vis_hid[:, hsl])),
                "vv_w": _permute_vis_rows(np.ascontiguousarray(vv[:, vsl])),
                "vht_w": np.ascontiguousarray(vht[:, vsl]),
                "hb_row": np.ascontiguousarray(hb[:, hsl]),
                "vb_row": np.ascontiguousarray(vb[:, vsl]),
                "ones_row": ones,
            }
        )
    return in_maps


def kernel(
    x,
    vis_bias,
    hid_bias,
    vis_hid,
    vis_vis_raw,
    hid_hid_raw,
    max_steps,
    _trace=False,
):
    from concourse import bass_utils

    n_steps = int(max_steps)
    steps_f = np.float32(n_steps)
    temps = (
        np.float32(0.01)
        * (
            np.float32(1.0)
            + np.float32(4.0)
            * np.exp(
                np.float32(-5.0)
                * np.arange(n_steps, dtype=np.float32)
                / steps_f
            )
        )
    ).astype(np.float32)

    if n_steps not in _BUILT:
        _BUILT[n_steps] = _build(n_steps, temps)
    nc = _BUILT[n_steps]

    in_maps = _prep_inputs(
        x, vis_bias, hid_bias, vis_hid, vis_vis_raw, hid_hid_raw
    )
    res = bass_utils.run_bass_kernel_spmd(
        nc, in_maps, core_ids=list(range(N_CORES)), trace=_trace
    )

    out = np.empty((BATCH, V_SIZE), dtype=np.float32)
    for c in range(N_CORES):
        out[:, c * VS : (c + 1) * VS] = res.results[c]["vis_shT"].T
    kernel._last_result = res
    return out



# revision 7
# speedup vs baseline: 1.0563x; 1.0082x over previous
"""Trainium2 Bass kernel for the annealed mean-field Boltzmann machine.

Strategy: 1D tensor-parallel over 8 NeuronCores. Each core holds a
256-column shard of hh/vis_hid and a 512-column shard of vv/vis_hid.T,
all SBUF-resident in fp32 (the dynamics are chaotic in the early
annealing steps, so reduced-precision matmuls diverge; fp32 matches the
reference to the level of summation-order noise).

States are kept transposed (feature-on-partition, batch-on-free).
Every field matmul uses the weight tile as the stationary operand
(128x128) and a state k-tile (128x64) as the moving operand:
out[feat_tile, batch] += W[k, feat_tile].T @ stateT[k]. Outputs come
out feature-major, exactly the layout the next step needs, so there are
no transposes anywhere. Bias enters as a rank-1 matmul (bias x ones),
sigmoid/(1/temp) on the scalar engine, 0.9/0.1 mixing on the vector
engine, and each core's state shard is AllGathered so every core has
the full state for the next half-step.
"""

import sys
import time

sys.path.insert(0, "/opt/trn_rl_repo")

import numpy as np

N_CORES = 8
V_SIZE = 4096
H_SIZE = 2048
BATCH = 64
HS = H_SIZE // N_CORES  # 256 hid cols per core
VS = V_SIZE // N_CORES  # 512 vis cols per core
KT_H = H_SIZE // 128  # 16 k-tiles over hid features
KT_V = V_SIZE // 128  # 32 k-tiles over vis features
NTH = HS // 128  # 2 feature out-tiles per core (hid)
NTV = VS // 128  # 4 feature out-tiles per core (vis)

_BUILT = {}


def _build(n_steps: int, temps: np.ndarray, sim_mode: bool = False,
           no_comm: bool = False, splits=(3, (4, 6, 6), (1, 1, 7, 7), (4, 6, 6))):
    import concourse.bacc as bacc
    import concourse.tile as tile
    import concourse.mybir as mybir

    F32 = mybir.dt.float32
    SIG = mybir.ActivationFunctionType.Sigmoid
    MULT = mybir.AluOpType.mult
    ADD = mybir.AluOpType.add

    nc = bacc.Bacc(
        "TRN2",
        target_bir_lowering=False,
        debug=False,
        enable_asserts=True,
        num_devices=1 if sim_mode else N_CORES,
    )

    def din(name, shape):
        return nc.dram_tensor(name, shape, F32, kind="ExternalInput").ap()

    xT = din("xT", [V_SIZE, BATCH])
    xT_my = din("xT_my", [VS, BATCH])
    hid0T = din("hid0T", [H_SIZE, BATCH])
    hh_w = din("hh_w", [H_SIZE, HS])
    vh_w = din("vh_w", [V_SIZE, HS])
    vv_w = din("vv_w", [V_SIZE, VS])
    vht_w = din("vht_w", [H_SIZE, VS])
    hb_row = din("hb_row", [1, HS])
    vb_row = din("vb_row", [1, VS])
    ones_row = din("ones_row", [1, BATCH])
    out_vis = nc.dram_tensor(
        "vis_shT", [VS, BATCH], F32, kind="ExternalOutput"
    ).ap()

    rg = [list(range(N_CORES))]
    shared_as = "Local" if sim_mode else "Shared"

    def all_gather(ag_in, ag_out, scratch):
        """Real AllGather, or in sim mode a 2-DMA pool-engine chain through a
        scratch DRAM tile: same ~5us latency and a single completion, without
        touching the engines/queues the real collective leaves free."""
        if not sim_mode:
            nc.gpsimd.collective_compute(
                "AllGather",
                mybir.AluOpType.bypass,
                replica_groups=rg,
                ins=[ag_in[:].opt()],
                outs=[ag_out[:].opt()],
            )
        else:
            rows = ag_in.shape[0]
            nc.gpsimd.dma_start(scratch[0:rows, :], ag_in[:])
            nc.gpsimd.dma_start(ag_out[:], scratch[:])

    with tile.TileContext(nc) as tc:
        with (
            tc.tile_pool(name="w", bufs=1) as wpool,
            tc.tile_pool(name="st", bufs=1) as stpool,
            tc.tile_pool(name="act", bufs=3) as actpool,
            tc.tile_pool(name="ps_h", bufs=2, space="PSUM") as ps_h,
            tc.tile_pool(name="ps_v", bufs=4, space="PSUM") as ps_v,
            tc.tile_pool(name="dram", bufs=2, space="DRAM") as dram,
        ):
            # --- constants + states first: tiny DMAs, so step 0's bias
            #     matmuls and first k-passes never queue behind 18 MiB of
            #     weights ---
            hb_sb = wpool.tile([1, HS], F32)
            vb_sb = wpool.tile([1, VS], F32)
            ones_sb = wpool.tile([1, BATCH], F32)
            nc.sync.dma_start(hb_sb[:], hb_row[:])
            nc.sync.dma_start(vb_sb[:], vb_row[:])
            nc.sync.dma_start(ones_sb[:], ones_row[:])

            visT = stpool.tile([128, KT_V, BATCH], F32)
            hidT = stpool.tile([128, KT_H, BATCH], F32)
            vmyT = stpool.tile([128, NTV, BATCH], F32)
            hmyT = stpool.tile([128, NTH, BATCH], F32)
            for j in range(0, KT_V, 8):
                nc.sync.dma_start(
                    visT[:, j : j + 8, :],
                    xT.rearrange("(k p) n -> p k n", p=128)[:, j : j + 8, :],
                )
            nc.sync.dma_start(vmyT[:], xT_my.rearrange("(k p) n -> p k n", p=128))
            nc.sync.dma_start(hidT[:], hid0T.rearrange("(k p) n -> p k n", p=128))
            nc.sync.dma_start(
                hmyT[:],
                hid0T.rearrange("(k p) n -> p k n", p=128)[:, :NTH, :],
            )

            # --- weights (SBUF-resident), blocked [k, j] 128x128, loaded in
            #     first-consumption order: hh, vh (hid field), vv, vht ---
            hh_sb = wpool.tile([128, KT_H, NTH, 128], F32)
            vh_sb = wpool.tile([128, KT_V, NTH, 128], F32)
            vv_sb = wpool.tile([128, KT_V, NTV, 128], F32)
            vht_sb = wpool.tile([128, KT_H, NTV, 128], F32)
            for j in range(0, KT_H, 4):
                nc.sync.dma_start(
                    hh_sb[:, j : j + 4, :, :],
                    hh_w.rearrange("(k p) (j n) -> p k j n", p=128, n=128)[
                        :, j : j + 4, :, :
                    ],
                )
            for j in range(0, KT_V, 4):
                nc.sync.dma_start(
                    vh_sb[:, j : j + 4, :, :],
                    vh_w.rearrange("(k p) (j n) -> p k j n", p=128, n=128)[
                        :, j : j + 4, :, :
                    ],
                )
            for j in range(0, KT_V, 4):
                nc.sync.dma_start(
                    vv_sb[:, j : j + 4, :, :],
                    vv_w.rearrange("(k p) (j n) -> p k j n", p=128, n=128)[
                        :, j : j + 4, :, :
                    ],
                )
            for j in range(0, KT_H, 4):
                nc.sync.dma_start(
                    vht_sb[:, j : j + 4, :, :],
                    vht_w.rearrange("(k p) (j n) -> p k j n", p=128, n=128)[
                        :, j : j + 4, :, :
                    ],
                )

            # vis and hid k-tiles are stored in AllGather output order
            # (host-side permutation), so restage is a contiguous copy and
            # consuming k in ascending order reads the early half first
            scr_h0 = dram.tile([H_SIZE, BATCH], F32, name="scr_h0", bufs=1)
            scr_h1 = dram.tile([H_SIZE // 2, BATCH], F32, name="scr_h1", bufs=1)
            scr_v0 = dram.tile([H_SIZE, BATCH], F32, name="scr_v0", bufs=1)
            scr_v1 = dram.tile([H_SIZE, BATCH], F32, name="scr_v1", bufs=1)
            scr_h = [scr_h0, scr_h1]
            scr_v = [scr_v0, scr_v1]
            KH2 = KT_H // 2  # 8: k-tiles per hid AllGather half
            KV2 = KT_V // 2  # 16: k-tiles per vis AllGather half
            # restage chunk ladder: small first chunk lands fast so the
            # first consumer matmuls start early
            RESTAGE_H = splits[1] if splits[1] else (2, 14)
            RESTAGE_V = splits[2] if splits[2] else (2, 14)
            RESTAGE_VB = splits[3] if len(splits) > 3 else RESTAGE_V

            for i in range(n_steps):
                inv_t = float(1.0 / temps[i])
                last = i == n_steps - 1

                # ---- field matmuls, ordered so that every comm chain is
                # covered by matmuls that do not depend on it ----
                phs = []
                for j in range(NTH):
                    ph = ps_h.tile(
                        [128, BATCH], F32, name=f"ph{i}_{j}", tag="ph"
                    )
                    phs.append(ph)
                    nc.tensor.matmul(
                        ph[:],
                        hb_sb[:, j * 128 : (j + 1) * 128],
                        ones_sb[:],
                        start=True,
                        stop=False,
                    )
                pvs = []
                for j in range(NTV):
                    pv = ps_v.tile(
                        [128, BATCH], F32, name=f"pv{i}_{j}", tag="pv"
                    )
                    pvs.append(pv)
                    nc.tensor.matmul(
                        pv[:],
                        vb_sb[:, j * 128 : (j + 1) * 128],
                        ones_sb[:],
                        start=True,
                        stop=False,
                    )
                # hid @ hh (needs hidT(i-1): available early)
                for k in range(KT_H):
                    for j in range(NTH):
                        nc.tensor.matmul(
                            phs[j][:], hh_sb[:, k, j, :], hidT[:, k, :],
                            start=False, stop=False,
                        )
                # vis-dependent matmuls, A-half k-tiles then B-half, with the
                # vv-A block between them as cover for the late AG_v_B
                for k in range(KV2):
                    for j in range(NTH):
                        nc.tensor.matmul(
                            phs[j][:], vh_sb[:, k, j, :], visT[:, k, :],
                            start=False, stop=False,
                        )
                VV_A = splits[0]
                for k in range(VV_A):
                    for j in range(NTV):
                        nc.tensor.matmul(
                            pvs[j][:], vv_sb[:, k, j, :], visT[:, k, :],
                            start=False, stop=False,
                        )
                for k in range(KV2, KT_V):
                    for j in range(NTH):
                        nc.tensor.matmul(
                            phs[j][:], vh_sb[:, k, j, :], visT[:, k, :],
                            start=False, stop=(k == KT_V - 1),
                        )
                # hid field complete: sigmoid + mix + AllGather
                for j in range(NTH):
                    ph = phs[j]
                    prob = actpool.tile(
                        [128, BATCH], F32, name=f"prh{i}_{j}", tag="pr"
                    )
                    nc.scalar.activation(prob[:], ph[:], SIG, scale=inv_t)
                    tmp = actpool.tile(
                        [128, BATCH], F32, name=f"tmh{i}_{j}", tag="tm"
                    )
                    nc.vector.tensor_sub(tmp[:], prob[:], hmyT[:, j, :])
                    nc.vector.scalar_tensor_tensor(
                        hmyT[:, j, :], tmp[:], 0.1, hmyT[:, j, :], MULT, ADD
                    )
                    if not no_comm:
                        if j == 0:
                            ag_in_h = dram.tile(
                                [HS, BATCH], F32, name=f"agih{i}", tag="agih"
                            )
                        nc.scalar.dma_start(
                            ag_in_h[j * 128 : (j + 1) * 128, :],
                            hmyT[:, j, :],
                        )
                if not no_comm:
                    ag_out_h = dram.tile(
                        [H_SIZE, BATCH], F32, addr_space=shared_as,
                        name=f"agoh{i}", tag="agoh",
                    )
                    all_gather(ag_in_h, ag_out_h, scr_h[0])
                    qs = 0
                    for w in RESTAGE_H:
                        nc.sync.dma_start(
                            hidT[:, qs : qs + w, :],
                            ag_out_h[:].rearrange("(k p) n -> p k n", p=128)[
                                :, qs : qs + w, :
                            ],
                        )
                        qs += w
                # finish vv for j0/j1 (covers the hid AllGather)
                for k in range(VV_A, KT_V):
                    for j in (0, 1):
                        nc.tensor.matmul(
                            pvs[j][:], vv_sb[:, k, j, :], visT[:, k, :],
                            start=False, stop=False,
                        )
                # hid(i)-dependent part of the vis field, then mix + AG per
                # j-pair; the j2/j3 vv tail runs between the two halves as
                # cover for AG_v_A
                for half in range(2):
                    js = (0, 1) if half == 0 else (2, 3)
                    for k in range(KT_H):
                        for j in js:
                            nc.tensor.matmul(
                                pvs[j][:], vht_sb[:, k, j, :], hidT[:, k, :],
                                start=False, stop=(k == KT_H - 1),
                            )
                    if half == 0:
                        for k in range(VV_A, KT_V):
                            for j in (2, 3):
                                nc.tensor.matmul(
                                    pvs[j][:], vv_sb[:, k, j, :], visT[:, k, :],
                                    start=False, stop=False,
                                )
                    for j in js:
                        pv = pvs[j]
                        prob = actpool.tile(
                            [128, BATCH], F32, name=f"prv{i}_{j}", tag="pr"
                        )
                        nc.scalar.activation(prob[:], pv[:], SIG, scale=inv_t)
                        tmp = actpool.tile(
                            [128, BATCH], F32, name=f"tmv{i}_{j}", tag="tm"
                        )
                        nc.vector.tensor_sub(tmp[:], prob[:], vmyT[:, j, :])
                        nc.vector.scalar_tensor_tensor(
                            vmyT[:, j, :], tmp[:], 0.1, vmyT[:, j, :], MULT, ADD
                        )
                        if not (last or no_comm):
                            if j % 2 == 0:
                                ag_in = dram.tile(
                                    [HS, BATCH], F32,
                                    name=f"agiv{i}_{half}", tag="agiv",
                                )
                            nc.scalar.dma_start(
                                ag_in[(j % 2) * 128 : (j % 2 + 1) * 128, :],
                                vmyT[:, j, :],
                            )
                    if last or no_comm:
                        continue
                    ag_out = dram.tile(
                        [H_SIZE, BATCH], F32, addr_space=shared_as,
                        name=f"agov{i}_{half}", tag="agov",
                    )
                    all_gather(ag_in, ag_out, scr_v[half])
                    qs = 0
                    for w in (RESTAGE_V if half == 0 else RESTAGE_VB):
                        nc.sync.dma_start(
                            visT[:, KV2 * half + qs : KV2 * half + qs + w, :],
                            ag_out[:].rearrange("(k p) n -> p k n", p=128)[
                                :, qs : qs + w, :
                            ],
                        )
                        qs += w

            nc.sync.dma_start(
                out_vis[:].rearrange("(k p) n -> p k n", p=128), vmyT[:]
            )

    nc.compile()
    return nc


# vis k-tile permutation: SBUF order k' = AllGather output order.
# k' in [0,16): half A = each core's feature tiles {0,1};  orig k = 4c+t
# k' in [16,32): half B = tiles {2,3};                      orig k = 4c+2+t
_PERM_V = [4 * (k % 16 // 2) + (2 * (k // 16)) + (k % 2) for k in range(32)]
# hid k-tile permutation: per-j AllGather j=0 gathers each core's tile 0
# (orig 2c) into k' = c, j=1 gathers tile 1 (orig 2c+1) into k' = 8+c
_PERM_H = [2 * k for k in range(8)] + [2 * k + 1 for k in range(8)]


def _permute_vis_rows(a):
    """Reorder 128-row blocks of a (4096, ...) array into gather order."""
    blocks = a.reshape(32, 128, *a.shape[1:])
    return np.ascontiguousarray(blocks[_PERM_V].reshape(a.shape))


def _permute_hid_rows(a):
    """Reorder 128-row blocks of a (2048, ...) array into gather order."""
    blocks = a.reshape(16, 128, *a.shape[1:])
    return np.ascontiguousarray(blocks[_PERM_H].reshape(a.shape))


def _prep_inputs(x, vis_bias, hid_bias, vis_hid, vis_vis_raw, hid_hid_raw):
    f32 = np.float32
    vv = np.triu(np.asarray(vis_vis_raw, dtype=f32), 1)
    vv = vv + vv.T
    hh = np.triu(np.asarray(hid_hid_raw, dtype=f32), 1)
    hh = hh + hh.T
    vis_hid = np.ascontiguousarray(np.asarray(vis_hid, dtype=f32))
    vht = np.ascontiguousarray(vis_hid.T)  # (H, V)
    x = np.asarray(x, dtype=f32)
    xT = np.ascontiguousarray(x.T)
    ones = np.ones((1, BATCH), dtype=f32)
    hid0 = np.full((H_SIZE, BATCH), 0.5, dtype=f32)
    hb = np.ascontiguousarray(np.asarray(hid_bias, dtype=f32).reshape(1, H_SIZE))
    vb = np.ascontiguousarray(np.asarray(vis_bias, dtype=f32).reshape(1, V_SIZE))

    in_maps = []
    for c in range(N_CORES):
        hsl = slice(c * HS, (c + 1) * HS)
        vsl = slice(c * VS, (c + 1) * VS)
        in_maps.append(
            {
                "xT": _permute_vis_rows(xT),
                "xT_my": np.ascontiguousarray(xT[vsl]),
                "hid0T": hid0,
                "hh_w": np.ascontiguousarray(hh[:, hsl]),
                "vh_w": _permute_vis_rows(np.ascontiguousarray(vis_hid[:, hsl])),
                "vv_w": _permute_vis_rows(np.ascontiguousarray(vv[:, vsl])),
                "vht_w": np.ascontiguousarray(vht[:, vsl]),
                "hb_row": np.ascontiguousarray(hb[:, hsl]),
                "vb_row": np.ascontiguousarray(vb[:, vsl]),
                "ones_row": ones,
            }
        )
    return in_maps


def kernel(
    x,
    vis_bias,
    hid_bias,
    vis_hid,
    vis_vis_raw,
    hid_hid_raw,
    max_steps,
    _trace=False,
):
    from concourse import bass_utils

    n_steps = int(max_steps)
    steps_f = np.float32(n_steps)
    temps = (
        np.float32(0.01)
        * (
            np.float32(1.0)
            + np.float32(4.0)
            * np.exp(
                np.float32(-5.0)
                * np.arange(n_steps, dtype=np.float32)
                / steps_f
            )
        )
    ).astype(np.float32)

    if n_steps not in _BUILT:
        _BUILT[n_steps] = _build(n_steps, temps)
    nc = _BUILT[n_steps]

    in_maps = _prep_inputs(
        x, vis_bias, hid_bias, vis_hid, vis_vis_raw, hid_hid_raw
    )
    res = bass_utils.run_bass_kernel_spmd(
        nc, in_maps, core_ids=list(range(N_CORES)), trace=_trace
    )

    out = np.empty((BATCH, V_SIZE), dtype=np.float32)
    for c in range(N_CORES):
        out[:, c * VS : (c + 1) * VS] = res.results[c]["vis_shT"].T
    kernel._last_result = res
    return out



# revision 8
# speedup vs baseline: 1.0576x; 1.0012x over previous
"""Trainium2 Bass kernel for the annealed mean-field Boltzmann machine.

Strategy: 1D tensor-parallel over 8 NeuronCores. Each core holds a
256-column shard of hh/vis_hid and a 512-column shard of vv/vis_hid.T,
all SBUF-resident in fp32 (the dynamics are chaotic in the early
annealing steps, so reduced-precision matmuls diverge; fp32 matches the
reference to the level of summation-order noise).

States are kept transposed (feature-on-partition, batch-on-free).
Every field matmul uses the weight tile as the stationary operand
(128x128) and a state k-tile (128x64) as the moving operand:
out[feat_tile, batch] += W[k, feat_tile].T @ stateT[k]. Outputs come
out feature-major, exactly the layout the next step needs, so there are
no transposes anywhere. Bias enters as a rank-1 matmul (bias x ones),
sigmoid/(1/temp) on the scalar engine, 0.9/0.1 mixing on the vector
engine, and each core's state shard is AllGathered so every core has
the full state for the next half-step.
"""

import sys
import time

sys.path.insert(0, "/opt/trn_rl_repo")

import numpy as np

N_CORES = 8
V_SIZE = 4096
H_SIZE = 2048
BATCH = 64
HS = H_SIZE // N_CORES  # 256 hid cols per core
VS = V_SIZE // N_CORES  # 512 vis cols per core
KT_H = H_SIZE // 128  # 16 k-tiles over hid features
KT_V = V_SIZE // 128  # 32 k-tiles over vis features
NTH = HS // 128  # 2 feature out-tiles per core (hid)
NTV = VS // 128  # 4 feature out-tiles per core (vis)

_BUILT = {}


def _build(n_steps: int, temps: np.ndarray, sim_mode: bool = False,
           no_comm: bool = False, splits=(3, (6, 5, 5), (1, 1, 7, 7), (4, 6, 6))):
    import concourse.bacc as bacc
    import concourse.tile as tile
    import concourse.mybir as mybir

    F32 = mybir.dt.float32
    SIG = mybir.ActivationFunctionType.Sigmoid
    MULT = mybir.AluOpType.mult
    ADD = mybir.AluOpType.add

    nc = bacc.Bacc(
        "TRN2",
        target_bir_lowering=False,
        debug=False,
        enable_asserts=True,
        num_devices=1 if sim_mode else N_CORES,
    )

    def din(name, shape):
        return nc.dram_tensor(name, shape, F32, kind="ExternalInput").ap()

    xT = din("xT", [V_SIZE, BATCH])
    xT_my = din("xT_my", [VS, BATCH])
    hid0T = din("hid0T", [H_SIZE, BATCH])
    hh_w = din("hh_w", [H_SIZE, HS])
    vh_w = din("vh_w", [V_SIZE, HS])
    vv_w = din("vv_w", [V_SIZE, VS])
    vht_w = din("vht_w", [H_SIZE, VS])
    hb_row = din("hb_row", [1, HS])
    vb_row = din("vb_row", [1, VS])
    ones_row = din("ones_row", [1, BATCH])
    out_vis = nc.dram_tensor(
        "vis_shT", [VS, BATCH], F32, kind="ExternalOutput"
    ).ap()

    rg = [list(range(N_CORES))]
    shared_as = "Local" if sim_mode else "Shared"

    def all_gather(ag_in, ag_out, scratch):
        """Real AllGather, or in sim mode a 2-DMA pool-engine chain through a
        scratch DRAM tile: same ~5us latency and a single completion, without
        touching the engines/queues the real collective leaves free."""
        if not sim_mode:
            nc.gpsimd.collective_compute(
                "AllGather",
                mybir.AluOpType.bypass,
                replica_groups=rg,
                ins=[ag_in[:].opt()],
                outs=[ag_out[:].opt()],
            )
        else:
            rows = ag_in.shape[0]
            nc.gpsimd.dma_start(scratch[0:rows, :], ag_in[:])
            nc.gpsimd.dma_start(ag_out[:], scratch[:])

    with tile.TileContext(nc) as tc:
        with (
            tc.tile_pool(name="w", bufs=1) as wpool,
            tc.tile_pool(name="st", bufs=1) as stpool,
            tc.tile_pool(name="act", bufs=3) as actpool,
            tc.tile_pool(name="ps_h", bufs=2, space="PSUM") as ps_h,
            tc.tile_pool(name="ps_v", bufs=4, space="PSUM") as ps_v,
            tc.tile_pool(name="dram", bufs=2, space="DRAM") as dram,
        ):
            # --- constants + states first: tiny DMAs, so step 0's bias
            #     matmuls and first k-passes never queue behind 18 MiB of
            #     weights ---
            hb_sb = wpool.tile([1, HS], F32)
            vb_sb = wpool.tile([1, VS], F32)
            ones_sb = wpool.tile([1, BATCH], F32)
            nc.sync.dma_start(hb_sb[:], hb_row[:])
            nc.sync.dma_start(vb_sb[:], vb_row[:])
            nc.sync.dma_start(ones_sb[:], ones_row[:])

            visT = stpool.tile([128, KT_V, BATCH], F32)
            hidT = stpool.tile([128, KT_H, BATCH], F32)
            vmyT = stpool.tile([128, NTV, BATCH], F32)
            hmyT = stpool.tile([128, NTH, BATCH], F32)
            for j in range(0, KT_V, 8):
                nc.sync.dma_start(
                    visT[:, j : j + 8, :],
                    xT.rearrange("(k p) n -> p k n", p=128)[:, j : j + 8, :],
                )
            nc.sync.dma_start(vmyT[:], xT_my.rearrange("(k p) n -> p k n", p=128))
            nc.sync.dma_start(hidT[:], hid0T.rearrange("(k p) n -> p k n", p=128))
            nc.sync.dma_start(
                hmyT[:],
                hid0T.rearrange("(k p) n -> p k n", p=128)[:, :NTH, :],
            )

            # --- weights (SBUF-resident), blocked [k, j] 128x128, loaded in
            #     first-consumption order: hh, vh (hid field), vv, vht ---
            hh_sb = wpool.tile([128, KT_H, NTH, 128], F32)
            vh_sb = wpool.tile([128, KT_V, NTH, 128], F32)
            vv_sb = wpool.tile([128, KT_V, NTV, 128], F32)
            vht_sb = wpool.tile([128, KT_H, NTV, 128], F32)
            for j in range(0, KT_H, 4):
                nc.sync.dma_start(
                    hh_sb[:, j : j + 4, :, :],
                    hh_w.rearrange("(k p) (j n) -> p k j n", p=128, n=128)[
                        :, j : j + 4, :, :
                    ],
                )
            for j in range(0, KT_V, 4):
                nc.sync.dma_start(
                    vh_sb[:, j : j + 4, :, :],
                    vh_w.rearrange("(k p) (j n) -> p k j n", p=128, n=128)[
                        :, j : j + 4, :, :
                    ],
                )
            for j in range(0, KT_V, 4):
                nc.sync.dma_start(
                    vv_sb[:, j : j + 4, :, :],
                    vv_w.rearrange("(k p) (j n) -> p k j n", p=128, n=128)[
                        :, j : j + 4, :, :
                    ],
                )
            for j in range(0, KT_H, 4):
                nc.sync.dma_start(
                    vht_sb[:, j : j + 4, :, :],
                    vht_w.rearrange("(k p) (j n) -> p k j n", p=128, n=128)[
                        :, j : j + 4, :, :
                    ],
                )

            # vis and hid k-tiles are stored in AllGather output order
            # (host-side permutation), so restage is a contiguous copy and
            # consuming k in ascending order reads the early half first
            scr_h0 = dram.tile([H_SIZE, BATCH], F32, name="scr_h0", bufs=1)
            scr_h1 = dram.tile([H_SIZE // 2, BATCH], F32, name="scr_h1", bufs=1)
            scr_v0 = dram.tile([H_SIZE, BATCH], F32, name="scr_v0", bufs=1)
            scr_v1 = dram.tile([H_SIZE, BATCH], F32, name="scr_v1", bufs=1)
            scr_h = [scr_h0, scr_h1]
            scr_v = [scr_v0, scr_v1]
            KH2 = KT_H // 2  # 8: k-tiles per hid AllGather half
            KV2 = KT_V // 2  # 16: k-tiles per vis AllGather half
            # restage chunk ladder: small first chunk lands fast so the
            # first consumer matmuls start early
            RESTAGE_H = splits[1] if splits[1] else (2, 14)
            RESTAGE_V = splits[2] if splits[2] else (2, 14)
            RESTAGE_VB = splits[3] if len(splits) > 3 else RESTAGE_V

            for i in range(n_steps):
                inv_t = float(1.0 / temps[i])
                last = i == n_steps - 1

                # ---- field matmuls, ordered so that every comm chain is
                # covered by matmuls that do not depend on it ----
                phs = []
                for j in range(NTH):
                    ph = ps_h.tile(
                        [128, BATCH], F32, name=f"ph{i}_{j}", tag="ph"
                    )
                    phs.append(ph)
                    nc.tensor.matmul(
                        ph[:],
                        hb_sb[:, j * 128 : (j + 1) * 128],
                        ones_sb[:],
                        start=True,
                        stop=False,
                    )
                pvs = []
                for j in range(NTV):
                    pv = ps_v.tile(
                        [128, BATCH], F32, name=f"pv{i}_{j}", tag="pv"
                    )
                    pvs.append(pv)
                    nc.tensor.matmul(
                        pv[:],
                        vb_sb[:, j * 128 : (j + 1) * 128],
                        ones_sb[:],
                        start=True,
                        stop=False,
                    )
                # hid @ hh (needs hidT(i-1): available early)
                for k in range(KT_H):
                    for j in range(NTH):
                        nc.tensor.matmul(
                            phs[j][:], hh_sb[:, k, j, :], hidT[:, k, :],
                            start=False, stop=False,
                        )
                # vis-dependent matmuls, A-half k-tiles then B-half, with the
                # vv-A block between them as cover for the late AG_v_B
                for k in range(KV2):
                    for j in range(NTH):
                        nc.tensor.matmul(
                            phs[j][:], vh_sb[:, k, j, :], visT[:, k, :],
                            start=False, stop=False,
                        )
                VV_A = splits[0]
                for k in range(VV_A):
                    for j in range(NTV):
                        nc.tensor.matmul(
                            pvs[j][:], vv_sb[:, k, j, :], visT[:, k, :],
                            start=False, stop=False,
                        )
                for k in range(KV2, KT_V):
                    for j in range(NTH):
                        nc.tensor.matmul(
                            phs[j][:], vh_sb[:, k, j, :], visT[:, k, :],
                            start=False, stop=(k == KT_V - 1),
                        )
                # hid field complete: sigmoid + mix + AllGather
                for j in range(NTH):
                    ph = phs[j]
                    prob = actpool.tile(
                        [128, BATCH], F32, name=f"prh{i}_{j}", tag="pr"
                    )
                    nc.scalar.activation(prob[:], ph[:], SIG, scale=inv_t)
                    tmp = actpool.tile(
                        [128, BATCH], F32, name=f"tmh{i}_{j}", tag="tm"
                    )
                    nc.vector.tensor_sub(tmp[:], prob[:], hmyT[:, j, :])
                    nc.vector.scalar_tensor_tensor(
                        hmyT[:, j, :], tmp[:], 0.1, hmyT[:, j, :], MULT, ADD
                    )
                    if not no_comm:
                        if j == 0:
                            ag_in_h = dram.tile(
                                [HS, BATCH], F32, name=f"agih{i}", tag="agih"
                            )
                        nc.scalar.dma_start(
                            ag_in_h[j * 128 : (j + 1) * 128, :],
                            hmyT[:, j, :],
                        )
                if not no_comm:
                    ag_out_h = dram.tile(
                        [H_SIZE, BATCH], F32, addr_space=shared_as,
                        name=f"agoh{i}", tag="agoh",
                    )
                    all_gather(ag_in_h, ag_out_h, scr_h[0])
                    qs = 0
                    for w in RESTAGE_H:
                        nc.sync.dma_start(
                            hidT[:, qs : qs + w, :],
                            ag_out_h[:].rearrange("(k p) n -> p k n", p=128)[
                                :, qs : qs + w, :
                            ],
                        )
                        qs += w
                # finish vv for j0/j1 (covers the hid AllGather)
                for k in range(VV_A, KT_V):
                    for j in (0, 1):
                        nc.tensor.matmul(
                            pvs[j][:], vv_sb[:, k, j, :], visT[:, k, :],
                            start=False, stop=False,
                        )
                # hid(i)-dependent part of the vis field, then mix + AG per
                # j-pair; the j2/j3 vv tail runs between the two halves as
                # cover for AG_v_A
                for half in range(2):
                    js = (0, 1) if half == 0 else (2, 3)
                    for k in range(KT_H):
                        for j in js:
                            nc.tensor.matmul(
                                pvs[j][:], vht_sb[:, k, j, :], hidT[:, k, :],
                                start=False, stop=(k == KT_H - 1),
                            )
                    if half == 0:
                        for k in range(VV_A, KT_V):
                            for j in (2, 3):
                                nc.tensor.matmul(
                                    pvs[j][:], vv_sb[:, k, j, :], visT[:, k, :],
                                    start=False, stop=False,
                                )
                    for j in js:
                        pv = pvs[j]
                        prob = actpool.tile(
                            [128, BATCH], F32, name=f"prv{i}_{j}", tag="pr"
                        )
                        nc.scalar.activation(prob[:], pv[:], SIG, scale=inv_t)
                        tmp = actpool.tile(
                            [128, BATCH], F32, name=f"tmv{i}_{j}", tag="tm"
                        )
                        nc.vector.tensor_sub(tmp[:], prob[:], vmyT[:, j, :])
                        nc.vector.scalar_tensor_tensor(
                            vmyT[:, j, :], tmp[:], 0.1, vmyT[:, j, :], MULT, ADD
                        )
                        if not (last or no_comm):
                            if j % 2 == 0:
                                ag_in = dram.tile(
                                    [HS, BATCH], F32,
                                    name=f"agiv{i}_{half}", tag="agiv",
                                )
                            nc.scalar.dma_start(
                                ag_in[(j % 2) * 128 : (j % 2 + 1) * 128, :],
                                vmyT[:, j, :],
                            )
                    if last or no_comm:
                        continue
                    ag_out = dram.tile(
                        [H_SIZE, BATCH], F32, addr_space=shared_as,
                        name=f"agov{i}_{half}", tag="agov",
                    )
                    all_gather(ag_in, ag_out, scr_v[half])
                    qs = 0
                    for w in (RESTAGE_V if half == 0 else RESTAGE_VB):
                        nc.sync.dma_start(
                            visT[:, KV2 * half + qs : KV2 * half + qs + w, :],
                            ag_out[:].rearrange("(k p) n -> p k n", p=128)[
                                :, qs : qs + w, :
                            ],
                        )
                        qs += w

            nc.sync.dma_start(
                out_vis[:].rearrange("(k p) n -> p k n", p=128), vmyT[:]
            )

    nc.compile()
    return nc


# vis k-tile permutation: SBUF order k' = AllGather output order.
# k' in [0,16): half A = each core's feature tiles {0,1};  orig k = 4c+t
# k' in [16,32): half B = tiles {2,3};                      orig k = 4c+2+t
_PERM_V = [4 * (k % 16 // 2) + (2 * (k // 16)) + (k % 2) for k in range(32)]
# hid k-tile permutation: per-j AllGather j=0 gathers each core's tile 0
# (orig 2c) into k' = c, j=1 gathers tile 1 (orig 2c+1) into k' = 8+c
_PERM_H = [2 * k for k in range(8)] + [2 * k + 1 for k in range(8)]


def _permute_vis_rows(a):
    """Reorder 128-row blocks of a (4096, ...) array into gather order."""
    blocks = a.reshape(32, 128, *a.shape[1:])
    return np.ascontiguousarray(blocks[_PERM_V].reshape(a.shape))


def _permute_hid_rows(a):
    """Reorder 128-row blocks of a (2048, ...) array into gather order."""
    blocks = a.reshape(16, 128, *a.shape[1:])
    return np.ascontiguousarray(blocks[_PERM_H].reshape(a.shape))


def _prep_inputs(x, vis_bias, hid_bias, vis_hid, vis_vis_raw, hid_hid_raw):
    f32 = np.float32
    vv = np.triu(np.asarray(vis_vis_raw, dtype=f32), 1)
    vv = vv + vv.T
    hh = np.triu(np.asarray(hid_hid_raw, dtype=f32), 1)
    hh = hh + hh.T
    vis_hid = np.ascontiguousarray(np.asarray(vis_hid, dtype=f32))
    vht = np.ascontiguousarray(vis_hid.T)  # (H, V)
    x = np.asarray(x, dtype=f32)
    xT = np.ascontiguousarray(x.T)
    ones = np.ones((1, BATCH), dtype=f32)
    hid0 = np.full((H_SIZE, BATCH), 0.5, dtype=f32)
    hb = np.ascontiguousarray(np.asarray(hid_bias, dtype=f32).reshape(1, H_SIZE))
    vb = np.ascontiguousarray(np.asarray(vis_bias, dtype=f32).reshape(1, V_SIZE))

    in_maps = []
    for c in range(N_CORES):
        hsl = slice(c * HS, (c + 1) * HS)
        vsl = slice(c * VS, (c + 1) * VS)
        in_maps.append(
            {
                "xT": _permute_vis_rows(xT),
                "xT_my": np.ascontiguousarray(xT[vsl]),
                "hid0T": hid0,
                "hh_w": np.ascontiguousarray(hh[:, hsl]),
                "vh_w": _permute_vis_rows(np.ascontiguousarray(vis_hid[:, hsl])),
                "vv_w": _permute_vis_rows(np.ascontiguousarray(vv[:, vsl])),
                "vht_w": np.ascontiguousarray(vht[:, vsl]),
                "hb_row": np.ascontiguousarray(hb[:, hsl]),
                "vb_row": np.ascontiguousarray(vb[:, vsl]),
                "ones_row": ones,
            }
        )
    return in_maps


def kernel(
    x,
    vis_bias,
    hid_bias,
    vis_hid,
    vis_vis_raw,
    hid_hid_raw,
    max_steps,
    _trace=False,
):
    from concourse import bass_utils

    n_steps = int(max_steps)
    steps_f = np.float32(n_steps)
    temps = (
        np.float32(0.01)
        * (
            np.float32(1.0)
            + np.float32(4.0)
            * np.exp(
                np.float32(-5.0)
                * np.arange(n_steps, dtype=np.float32)
                / steps_f
            )
        )
    ).astype(np.float32)

    if n_steps not in _BUILT:
        _BUILT[n_steps] = _build(n_steps, temps)
    nc = _BUILT[n_steps]

    in_maps = _prep_inputs(
        x, vis_bias, hid_bias, vis_hid, vis_vis_raw, hid_hid_raw
    )
    res = bass_utils.run_bass_kernel_spmd(
        nc, in_maps, core_ids=list(range(N_CORES)), trace=_trace
    )

    out = np.empty((BATCH, V_SIZE), dtype=np.float32)
    for c in range(N_CORES):
        out[:, c * VS : (c + 1) * VS] = res.results[c]["vis_shT"].T
    kernel._last_result = res
    return out

